# revision 3
# baseline (speedup 1.0000x reference)
"""Trainium2 Bass kernel for nn_JslBERT — v2: fp8 DoubleRow Q/K/scores.

Sharding: 8 cores = 4 batch x 2 head-groups (6 heads each). Per layer,
attention-output partials are pairwise AllReduced in bf16; LN+FFN run
redundantly on both cores of a pair.

v2 changes vs baseline:
 - Q-proj, K-proj and QK^T scores run in fp8(e4m3) with perf_mode=DoubleRow
   (contraction pairs of 128-chunks per instruction). Static power-of-2
   scaling: weights x1024, residual x16 (x512 in layer 0), Q^T/K^T stored
   x32; descale folded into the PSUM drains and the exp() scale.
   V/ctx/out-proj/FFN stay bf16 (precision-critical path; fp8 there blows
   the 2e-2 gate).
 - Weight DMAs consolidated: one DMA per (tensor, head[, chunk]) instead of
   one per 128-row d-chunk (HWDGE instruction-count was near saturation).
"""
import os
import numpy as np
import ml_dtypes

import concourse.bass as bass
import concourse.bacc as bacc
import concourse.tile as tile
import concourse.bass_utils as bass_utils
from concourse import mybir
from concourse.masks import make_identity

# Model dims (hardcoded per problem spec)
B, S, L, D, H, V, PMAX = 4, 512, 4, 768, 12, 32000, 512
EPS = 1e-3
NCORES = 8
HPC = H // 2          # heads per core
KH = D                # head dim (768)
HK = HPC * KH         # 4608 flattened head dims per core
SCALE = 1.0 / float(np.sqrt(D))

F32 = mybir.dt.float32
BF16 = mybir.dt.bfloat16
F8 = mybir.dt.float8e4
I32 = mybir.dt.int32
DR = mybir.MatmulPerfMode.DoubleRow

TT = S // 128         # 4 t-tiles total
DC = D // 128         # 6 d-chunks
JD = DC // 2          # 3 d-chunk pairs (DoubleRow)
NCH = [(0, 512), (512, 256)]  # free-dim chunks for width-768 outputs
NTC = 2               # t-chunks per sequence
TPC = TT // NTC       # 128-tiles per chunk (2)
CW = S // NTC         # chunk width (256)

# fp8 static scales
W_SC = 1024.0                     # wq/wk weights
QK_SC = 32.0                      # stored Q^T / K^T
EXP_SCALE = SCALE / (QK_SC * QK_SC)


def a_sc(li):                     # residual-stream fp8 scale
    return 512.0 if li == 0 else 16.0


def qk_drain(li):                 # PSUM -> qt/kt fp8 multiplier
    return QK_SC / (a_sc(li) * W_SC)


BF = np.dtype(ml_dtypes.bfloat16)
E4 = np.dtype(ml_dtypes.float8_e4m3)


def build_nc(n_layers=L, flags=None):
    """Build the Bass graph. flags: dict of which optional inputs exist."""
    flags = flags or {}
    nc = bacc.Bacc("TRN2", target_bir_lowering=False, debug=False,
                   num_devices=NCORES)

    xids_d = nc.dram_tensor("xids", [3, S], I32, kind="ExternalInput").ap()
    tokw_d = nc.dram_tensor("tok_w", [V, D], BF16, kind="ExternalInput").ap()
    posw_d = nc.dram_tensor("pos_w", [PMAX, D], BF16, kind="ExternalInput").ap()
    segw_d = nc.dram_tensor("seg_w", [2, D], BF16, kind="ExternalInput").ap()
    # fp8 pair layouts: [L, h, p, j, i, 768] with slot i = d-chunk 2j+i
    wq_d = nc.dram_tensor("wq8", [n_layers, HPC, 128, JD, 2, KH], F8, kind="ExternalInput").ap()
    wk_d = nc.dram_tensor("wk8", [n_layers, HPC, 128, JD, 2, KH], F8, kind="ExternalInput").ap()
    # bf16 per-head layouts: [L, h, p, dc|kc, 768]
    wv_d = nc.dram_tensor("wv", [n_layers, HPC, 128, DC, KH], BF16, kind="ExternalInput").ap()
    wo_d = nc.dram_tensor("wo", [n_layers, HPC, 128, DC, D], BF16, kind="ExternalInput").ap()
    ff_d = nc.dram_tensor("ff", [n_layers, 128, DC, D], BF16, kind="ExternalInput").ap()
    out_d = nc.dram_tensor("out", [S, D], F32, kind="ExternalOutput").ap()

    opt = {}
    if flags.get("emb_bias"):
        opt["emb_bias"] = nc.dram_tensor("emb_bias", [D], F32, kind="ExternalInput").ap()
    for nm in ("ln1", "ln2"):
        if flags.get(nm):
            opt[nm + "_g"] = nc.dram_tensor(nm + "_g", [n_layers, D], F32, kind="ExternalInput").ap()
            opt[nm + "_b"] = nc.dram_tensor(nm + "_b", [n_layers, D], F32, kind="ExternalInput").ap()
    if flags.get("mask"):
        opt["maskneg"] = nc.dram_tensor("maskneg", [S], F32, kind="ExternalInput").ap()

    with tile.TileContext(nc) as tc:
        import contextlib
        with contextlib.ExitStack() as ctx:
            _build_body(ctx, tc, n_layers, flags, xids_d, tokw_d, posw_d, segw_d,
                        wq_d, wk_d, wv_d, wo_d, ff_d, out_d, opt)
    nc.compile()
    return nc


def _build_body(ctx, tc, n_layers, flags, xids_d, tokw_d, posw_d, segw_d,
                wq_d, wk_d, wv_d, wo_d, ff_d, out_d, opt):
    nc = tc.nc

    const = ctx.enter_context(tc.tile_pool(name="const", bufs=1))
    wq8_pool = ctx.enter_context(tc.tile_pool(name="wq8", bufs=3))
    wk8_pool = ctx.enter_context(tc.tile_pool(name="wk8", bufs=3))
    wbig_pool = ctx.enter_context(tc.tile_pool(name="wbig", bufs=5))   # wv/wo
    ff_pool = ctx.enter_context(tc.tile_pool(name="ffp", bufs=1))
    rT8_pool = ctx.enter_context(tc.tile_pool(name="rT8", bufs=8))
    rT_pool = ctx.enter_context(tc.tile_pool(name="rT", bufs=13))
    kt_pool = ctx.enter_context(tc.tile_pool(name="ktp", bufs=18))
    v_pool = ctx.enter_context(tc.tile_pool(name="vp", bufs=24))
    qt_pool = ctx.enter_context(tc.tile_pool(name="qtp", bufs=19))
    pe_pool = ctx.enter_context(tc.tile_pool(name="pep", bufs=5))
    pt_pool = ctx.enter_context(tc.tile_pool(name="ptp", bufs=8))
    ct_pool = ctx.enter_context(tc.tile_pool(name="ctp", bufs=5))
    xtd_pool = ctx.enter_context(tc.tile_pool(name="xtd", bufs=6))
    accb_pool = ctx.enter_context(tc.tile_pool(name="accb", bufs=4))
    sm_pool = ctx.enter_context(tc.tile_pool(name="sm", bufs=8))
    ps_mm = ctx.enter_context(tc.tile_pool(name="psmm", bufs=6, space="PSUM"))
    ps_tp = ctx.enter_context(tc.tile_pool(name="pstp", bufs=2, space="PSUM"))
    dram = ctx.enter_context(tc.tile_pool(name="dram", bufs=2, space="DRAM"))

    ident = const.tile([128, 128], F32)
    make_identity(nc, ident[:])
    identb = const.tile([128, 128], BF16)
    make_identity(nc, identb[:])
    eps_t = const.tile([128, 1], F32)
    nc.vector.memset(eps_t[:], EPS)

    def mm_tile():
        return ps_mm.tile([128, 512], F32, tag="mm", name="mmps")

    # ---- weight loaders ------------------------------------------------
    def load_wq8(li, h):
        t = wq8_pool.tile([128, JD, 2, KH], F8, tag="wq8")
        nc.sync.dma_start(t[:], wq_d[li, h])
        return t

    def load_wk8(li, h):
        t = wk8_pool.tile([128, JD, 2, KH], F8, tag="wk8")
        nc.sync.dma_start(t[:], wk_d[li, h])
        return t

    def load_big(wd, li, h=None):
        pool = wbig_pool if h is not None else ff_pool
        t = pool.tile([128, DC, D], BF16, tag="wbig" if h is not None else "ff")
        nc.sync.dma_start(t[:], wd[li] if h is None else wd[li, h])
        return t

    # ---- embeddings ----------------------------------------------------
    idx = const.tile([128, 3, TT], I32)
    nc.sync.dma_start(idx[:], xids_d.rearrange("k (j p) -> p k j", p=128))

    emb_bias_ap = None
    if "emb_bias" in opt:
        eb = const.tile([128, DC], F32)
        nc.sync.dma_start(eb[:], opt["emb_bias"].rearrange("(c p) -> p c", p=128))
        emb_bias_ap = [eb[:, c:c + 1] for c in range(DC)]

    # pos ids are structurally arange(S) (built that way in the model), so the
    # pos "lookup" is a direct row DMA; tok/seg stay data-dependent gathers.
    tok_t = wbig_pool.tile([128, DC, D], BF16, tag="wbig", name="tokt")
    pos_t = wbig_pool.tile([128, DC, D], BF16, tag="wbig", name="post")
    seg_t = wbig_pool.tile([128, DC, D], BF16, tag="wbig", name="segt")
    nc.sync.dma_start(pos_t[:, 0:TT, :],
                      posw_d.rearrange("(a p) d -> p a d", p=128))
    for tm in range(TT):
        nc.gpsimd.indirect_dma_start(
            out=tok_t[:, tm, :], out_offset=None, in_=tokw_d[:],
            in_offset=bass.IndirectOffsetOnAxis(ap=idx[:, 0, tm:tm + 1], axis=0))
        nc.gpsimd.indirect_dma_start(
            out=seg_t[:, tm, :], out_offset=None, in_=segw_d[:],
            in_offset=bass.IndirectOffsetOnAxis(ap=idx[:, 2, tm:tm + 1], axis=0))
    for tm in range(TT):
        nc.vector.tensor_add(tok_t[:, tm, :], tok_t[:, tm, :], pos_t[:, tm, :])
        nc.vector.tensor_add(tok_t[:, tm, :], tok_t[:, tm, :], seg_t[:, tm, :])

    # resTc[tcix][dc]: [128 d, 256 t] bf16 (V-proj stationary)
    # rT8c[tcix][j]:  [128 d, 2, 256 t] fp8 pairs (Q/K moving operand)
    resTc = [[None] * DC for _ in range(NTC)]
    rT8c = [[None] * JD for _ in range(NTC)]
    for tcix in range(NTC):
        for dc in range(DC):
            pp = ps_tp.tile([128, CW], BF16, tag="tp", name="tpps")
            for tl in range(TPC):
                nc.tensor.transpose(pp[:, tl * 128:(tl + 1) * 128],
                                    tok_t[:, tcix * TPC + tl, dc * 128:(dc + 1) * 128],
                                    identb[:])
            rt = rT_pool.tile([128, CW], BF16, tag="rT")
            if emb_bias_ap is not None:
                nc.vector.tensor_scalar_add(rt[:], pp[:], emb_bias_ap[dc])
            else:
                nc.vector.tensor_copy(out=rt[:], in_=pp[:])
            resTc[tcix][dc] = rt
            if dc % 2 == 0:
                rT8c[tcix][dc // 2] = rT8_pool.tile([128, 2, CW], F8, tag="rT8", name="rt8e")
            if emb_bias_ap is not None:
                nc.vector.tensor_scalar(out=rT8c[tcix][dc // 2][:, dc % 2, :],
                                        in0=pp[:], scalar1=emb_bias_ap[dc],
                                        scalar2=a_sc(0),
                                        op0=mybir.AluOpType.add,
                                        op1=mybir.AluOpType.mult)
            else:
                nc.vector.tensor_scalar_mul(rT8c[tcix][dc // 2][:, dc % 2, :],
                                            pp[:], a_sc(0))

    mask_ap = None
    if "maskneg" in opt:
        mk = const.tile([128, S], F32)
        nc.sync.dma_start(mk[:], opt["maskneg"].partition_broadcast(128))
        mask_ap = mk

    # ---- per-layer helpers --------------------------------------------
    QT_AHEAD = 6  # heads whose chunk-0 QT is prefetched at the end of the previous layer

    def load_ln_gb(li, nm):
        if nm + "_g" not in opt:
            return None
        gb = const.tile([128, 2, D], F32, tag=f"lngb{nm}{li}")
        nc.sync.dma_start(gb[:, 0, :], opt[nm + "_g"][li].partition_broadcast(128))
        nc.sync.dma_start(gb[:, 1, :], opt[nm + "_b"][li].partition_broadcast(128))
        return gb

    def layernorm(aps, gb):
        for x in aps:
            stats = sm_pool.tile([128, 3, 6], F32, tag="bnst")
            mv = sm_pool.tile([128, 2], F32, tag="bnmv")
            xg = x.rearrange("p (a c) -> p a c", a=3)
            for a in range(3):
                nc.vector.bn_stats(out=stats[:, a, :], in_=xg[:, a, :])
            nc.vector.bn_aggr(out=mv[:], in_=stats[:])
            rstd = sm_pool.tile([128, 1], F32, tag="rstd")
            nc.scalar.activation(out=rstd[:], in_=mv[:, 1:2],
                                 func=mybir.ActivationFunctionType.Sqrt,
                                 bias=eps_t[:], scale=1.0)
            nc.vector.reciprocal(rstd[:], rstd[:])
            nc.vector.tensor_scalar(out=x, in0=x, scalar1=mv[:, 0:1],
                                    scalar2=rstd[:],
                                    op0=mybir.AluOpType.subtract,
                                    op1=mybir.AluOpType.mult)
            if gb is not None:
                nc.vector.tensor_mul(x, x, gb[:, 0, :])
                nc.vector.tensor_add(x, x, gb[:, 1, :])

    def emit_kv_half(li, sc, h, rT8_l, resTc_l, kt_all, v_all):
        """KT s-half (fp8 DoubleRow) + V s-half (bf16) for one head.
        kt[(h,j)]: [128 k, 2, 512 s] fp8 pairs, v[(h,sm)]: [128 s, 768 k] bf16."""
        wk8 = load_wk8(li, h)
        wv_t = load_big(wv_d, li, h)
        dr_sc = qk_drain(li)
        for m2 in range(JD):
            pm = mm_tile()
            for half in range(2):
                m = 2 * m2 + half
                for j in range(JD):
                    nc.tensor.matmul(pm[:, half * CW:half * CW + CW],
                                     wk8[:, j, :, m * 128:(m + 1) * 128],
                                     rT8_l[sc][j][:],
                                     start=(j == 0), stop=(j == JD - 1),
                                     perf_mode=DR)
            if sc == 0:
                kt_all[(h, m2)] = kt_pool.tile([128, 2, S], F8, tag="kt",
                                               name=f"kt{h}_{m2}")
            pmv = pm[:].rearrange("p (i c) -> p i c", i=2)
            if m2 % 2 == 0:
                nc.scalar.mul(kt_all[(h, m2)][:, :, sc * CW:(sc + 1) * CW],
                              pmv, dr_sc)
            else:
                nc.vector.tensor_scalar_mul(
                    kt_all[(h, m2)][:, :, sc * CW:(sc + 1) * CW],
                    pmv, dr_sc)
        for tl in range(TPC):
            sm = sc * TPC + tl
            vt = v_pool.tile([128, D], BF16, tag="v")
            v_all[(h, sm)] = vt
            for (n0, nw) in NCH:
                pm = mm_tile()
                for dc in range(DC):
                    nc.tensor.matmul(pm[:, :nw],
                                     resTc_l[sc][dc][:, tl * 128:(tl + 1) * 128],
                                     wv_t[:, dc, n0:n0 + nw],
                                     start=(dc == 0), stop=(dc == DC - 1))
                nc.scalar.copy(out=vt[:, n0:n0 + nw], in_=pm[:, :nw])

    def emit_qt(li, tcix, h, rT8_l):
        """QT for one head/chunk via fp8 DoubleRow, packed 2 m's per PSUM bank.
        Returns 3 tiles [128, 512] fp8: tile j = m (2j, 2j+1) x 256 t."""
        wq8 = load_wq8(li, h)
        dr_sc = qk_drain(li)
        qt_sb = []
        for jo in range(JD):
            pm = mm_tile()
            for half in range(2):
                m = 2 * jo + half
                for j in range(JD):
                    nc.tensor.matmul(pm[:, half * CW:half * CW + CW],
                                     wq8[:, j, :, m * 128:(m + 1) * 128],
                                     rT8_l[tcix][j][:],
                                     start=(j == 0), stop=(j == JD - 1),
                                     perf_mode=DR)
            ot = qt_pool.tile([128, 512], F8, tag="qt")
            if jo % 2 == 0:
                nc.vector.tensor_scalar_mul(ot[:], pm[:], dr_sc)
            else:
                nc.scalar.mul(ot[:], pm[:], dr_sc)
            qt_sb.append(ot)
        return qt_sb

    def emit_scores(li, tcix, h, qt_sb, kt_all):
        """scores (fp8 DoubleRow) + exp + 1/sum diag for one head/chunk."""
        pe_list, diag_list = [], []
        for tl in range(TPC):
            pm = mm_tile()
            for j in range(JD):
                qv = qt_sb[j][:].rearrange("p (i c) -> p i c", i=2)
                nc.tensor.matmul(pm[:],
                                 qv[:, :, tl * 128:(tl + 1) * 128],
                                 kt_all[(h, j)][:],
                                 start=(j == 0), stop=(j == JD - 1),
                                 perf_mode=DR)
            if mask_ap is not None:
                nc.vector.tensor_add(pm[:], pm[:], mask_ap[:])
            pe = pe_pool.tile([128, S], BF16, tag="pe")
            sums = sm_pool.tile([128, 1], F32, tag="sums")
            nc.scalar.activation(out=pe[:], in_=pm[:],
                                 func=mybir.ActivationFunctionType.Exp,
                                 scale=EXP_SCALE, accum_out=sums[:])
            rec = sm_pool.tile([128, 1], F32, tag="rec")
            nc.vector.reciprocal(rec[:], sums[:])
            dg = sm_pool.tile([128, 128], BF16, tag="diag")
            nc.vector.tensor_scalar_mul(dg[:], identb[:], rec[:])
            pe_list.append(pe)
            diag_list.append(dg)
        return pe_list, diag_list

    def emit_ptco(li, tcix, h, pe_list, diag_list, v_all, acc, accb):
        """normalized P^T, ctxT, out-partial accumulate for one head/chunk."""
        # PT packed: tile j holds s-tiles (2j | 2j+1) x [2 tl x 128]
        pt_sb = []
        for j in range(TT // 2):
            pp = mm_tile()
            for half in range(2):
                sm = 2 * j + half
                for tl in range(TPC):
                    nc.tensor.matmul(pp[:, half * CW + tl * 128:half * CW + (tl + 1) * 128],
                                     pe_list[tl][:, sm * 128:(sm + 1) * 128],
                                     diag_list[tl][:], start=True, stop=True)
            ps = pt_pool.tile([128, 512], BF16, tag="pts")
            nc.vector.tensor_copy(out=ps[:], in_=pp[:])
            pt_sb.append(ps)

        # ctxT packed: tile j holds km (2j | 2j+1) x 256 t
        ct_sb = []
        for j in range(DC // 2):
            pm = mm_tile()
            for half in range(2):
                km = 2 * j + half
                for sm in range(TT):
                    nc.tensor.matmul(pm[:, half * CW:half * CW + CW],
                                     v_all[(h, sm)][:, km * 128:(km + 1) * 128],
                                     pt_sb[sm // 2][:, (sm % 2) * CW:(sm % 2) * CW + CW],
                                     start=(sm == 0), stop=(sm == TT - 1))
            ot = ct_pool.tile([128, 512], BF16, tag="ct")
            nc.vector.tensor_copy(out=ot[:], in_=pm[:])
            ct_sb.append(ot)

        wo_t = load_big(wo_d, li, h)
        for tl in range(TPC):
            for (n0, nw) in NCH:
                pm = mm_tile()
                for kc in range(DC):
                    nc.tensor.matmul(pm[:, :nw],
                                     ct_sb[kc // 2][:, (kc % 2) * CW + tl * 128:
                                                    (kc % 2) * CW + (tl + 1) * 128],
                                     wo_t[:, kc, n0:n0 + nw],
                                     start=(kc == 0), stop=(kc == DC - 1))
                if h == 0:
                    nc.vector.tensor_copy(out=acc[tl][:, n0:n0 + nw],
                                          in_=pm[:, :nw])
                elif h < HPC - 1:
                    nc.vector.tensor_add(acc[tl][:, n0:n0 + nw],
                                         acc[tl][:, n0:n0 + nw], pm[:, :nw])
                else:
                    nc.vector.tensor_add(accb[:, tl, n0:n0 + nw],
                                         acc[tl][:, n0:n0 + nw], pm[:, :nw])

    def emit_collective(li, accb):
        arin = dram.tile([CW, D], BF16, tag="arin")
        last = li == n_layers - 1
        nc.sync.dma_start(arin[:].rearrange("(a p) d -> p a d", p=128),
                          accb[:, 0:TPC, :])
        if last:
            arout = dram.tile([128, D], BF16, tag="arout2")
            nc.gpsimd.collective_compute(
                "ReduceScatter", mybir.AluOpType.add,
                replica_groups=[[0, 1], [2, 3], [4, 5], [6, 7]],
                ins=[arin.opt()], outs=[arout.opt()])
        else:
            # AllGather both partials (no AllReduce cost multiplier on the
            # collective cores); the pairwise add happens locally on DVE.
            arout = dram.tile([2 * CW, D], BF16, tag="arout")
            nc.gpsimd.collective_compute(
                "AllGather", mybir.AluOpType.bypass,
                replica_groups=[[0, 1], [2, 3], [4, 5], [6, 7]],
                ins=[arin.opt()], outs=[arout.opt()])
        return arout

    def emit_tail_chunk(li, tcix, arout, gb1, gb2, ff_t, resTc_next, rT8_next):
        """AR result -> LN1 -> FFN -> LN2 -> resTc_next[tcix] (or output DMA).
        For the last layer the collective was a ReduceScatter: each core owns
        128 of the 256 chunk rows; the host reassembles."""
        ntl = 1 if li == n_layers - 1 else TPC
        xc = accb_pool.tile([128, TPC, D], BF16, tag="accb", name="xcur")
        if li == n_layers - 1:
            nc.gpsimd.dma_start(xc[:, 0:ntl, :],
                                arout[:].rearrange("(a p) d -> p a d", p=128))
        else:
            xg = wbig_pool.tile([128, 2, TPC, D], BF16, tag="wbig", name="xg")
            nc.gpsimd.dma_start(xg[:],
                                arout[:].rearrange("(g a p) d -> p g a d", p=128, g=2))
            nc.vector.tensor_add(xc[:, 0:ntl, :], xg[:, 0, :, :], xg[:, 1, :, :])
        xcur = [xc[:, tl, :] for tl in range(ntl)]
        layernorm(xcur, gb1)

        lnT = []
        for dc in range(DC):
            pp = ps_tp.tile([128, CW], BF16, tag="tp", name="tpps")
            for tl in range(ntl):
                nc.tensor.transpose(pp[:, tl * 128:(tl + 1) * 128],
                                    xc[:, tl, dc * 128:(dc + 1) * 128],
                                    identb[:])
            t = pt_pool.tile([128, 512], BF16, tag="pts", name="lnT")
            nc.scalar.copy(out=t[:, :ntl * 128], in_=pp[:, :ntl * 128])
            lnT.append(t)

        xmid = [xtd_pool.tile([128, D], F32, tag="xtd", name=f"xmid{tl}") for tl in range(ntl)]
        xmid_ap = [t[:] for t in xmid]
        for tl in range(ntl):
            for (n0, nw) in NCH:
                pm = mm_tile()
                for dc in range(DC):
                    nc.tensor.matmul(pm[:, :nw], lnT[dc][:, tl * 128:(tl + 1) * 128],
                                     ff_t[:, dc, n0:n0 + nw],
                                     start=(dc == 0), stop=(dc == DC - 1))
                nc.vector.tensor_copy(out=xmid[tl][:, n0:n0 + nw], in_=pm[:, :nw])

        layernorm(xmid_ap, gb2)

        if li < n_layers - 1:
            for dc in range(DC):
                pp = ps_tp.tile([128, CW], F32, tag="tp", name="tpps")
                for tl in range(TPC):
                    nc.tensor.transpose(pp[:, tl * 128:(tl + 1) * 128],
                                        xmid[tl][:, dc * 128:(dc + 1) * 128],
                                        ident[:])
                rt = rT_pool.tile([128, CW], BF16, tag="rT")
                nc.scalar.copy(out=rt[:], in_=pp[:])
                resTc_next[tcix][dc] = rt
                if dc % 2 == 0:
                    rT8_next[tcix][dc // 2] = rT8_pool.tile([128, 2, CW], F8, tag="rT8", name="rt8n")
                nc.vector.tensor_scalar_mul(rT8_next[tcix][dc // 2][:, dc % 2, :],
                                            pp[:], a_sc(li + 1))
        else:
            nc.sync.dma_start(out_d[tcix * 128:(tcix + 1) * 128, :], xmid[0][:])

    # ---- layers --------------------------------------------------------
    # prologue: layer-0 chunk-0 KV + QT prefetch (resTc from embeddings)
    cur_kt, cur_v = {}, {}
    qt_pre = {}
    for h in range(HPC):
        emit_kv_half(0, 0, h, rT8c, resTc, cur_kt, cur_v)
        if h < QT_AHEAD:
            qt_pre[h] = emit_qt(0, 0, h, rT8c)

    for li in range(n_layers):
        gb1 = load_ln_gb(li, "ln1")
        gb2 = load_ln_gb(li, "ln2")

        # A: KV s-half 1 (skewed) + chunk-0 score chains
        acc0 = [xtd_pool.tile([128, D], F32, tag="xtd", name=f"acc{tl}") for tl in range(TPC)]
        accb0 = accb_pool.tile([128, TPC, D], BF16, tag="accb", name="accb0")
        emit_kv_half(li, 1, 0, rT8c, resTc, cur_kt, cur_v)
        pend = None
        for h in range(HPC):
            if h + 1 < HPC:
                emit_kv_half(li, 1, h + 1, rT8c, resTc, cur_kt, cur_v)
            qt_sb = qt_pre.pop(h) if h in qt_pre else emit_qt(li, 0, h, rT8c)
            sc_out = emit_scores(li, 0, h, qt_sb, cur_kt)
            if pend is not None:
                emit_ptco(li, 0, pend[0], pend[1], pend[2], cur_v, acc0, accb0)
            pend = (h, sc_out[0], sc_out[1])
        emit_ptco(li, 0, pend[0], pend[1], pend[2], cur_v, acc0, accb0)
        arout0 = emit_collective(li, accb0)

        # C: chunk-1 score chains (AR(c0) overlaps this)
        acc1 = [xtd_pool.tile([128, D], F32, tag="xtd", name=f"acc{tl}") for tl in range(TPC)]
        accb1 = accb_pool.tile([128, TPC, D], BF16, tag="accb", name="accb1")
        pend = None
        for h in range(HPC):
            qt_sb = emit_qt(li, 1, h, rT8c)
            sc_out = emit_scores(li, 1, h, qt_sb, cur_kt)
            if pend is not None:
                emit_ptco(li, 1, pend[0], pend[1], pend[2], cur_v, acc1, accb1)
            pend = (h, sc_out[0], sc_out[1])
        emit_ptco(li, 1, pend[0], pend[1], pend[2], cur_v, acc1, accb1)

        ff_t = load_big(ff_d, li)

        # E: tail chunk 0 — emitted before the chunk-1 collective so its
        # gathered-load/add/LN serial chain runs during C on the free engines
        resTc_next = [[None] * DC for _ in range(NTC)]
        rT8_next = [[None] * JD for _ in range(NTC)]
        emit_tail_chunk(li, 0, arout0, gb1, gb2, ff_t, resTc_next, rT8_next)
        arout1 = emit_collective(li, accb1)

        # F: next layer's chunk-0 KV + QT prefetch (fills AR(c1) window)
        next_kt, next_v = {}, {}
        qt_pre = {}
        if li < n_layers - 1:
            for h in range(HPC):
                emit_kv_half(li + 1, 0, h, rT8_next, resTc_next, next_kt, next_v)
                if h < QT_AHEAD:
                    qt_pre[h] = emit_qt(li + 1, 0, h, rT8_next)

        # G: tail chunk 1
        emit_tail_chunk(li, 1, arout1, gb1, gb2, ff_t, resTc_next, rT8_next)

        resTc = resTc_next
        rT8c = rT8_next
        cur_kt, cur_v = next_kt, next_v


# ------------------------------------------------------------------------
# host side
# ------------------------------------------------------------------------
_CACHED = {}
_LAST_RES = None


def _get_nc(n_layers, flag_key, flags):
    key = (n_layers, flag_key)
    if key not in _CACHED:
        _CACHED[key] = build_nc(n_layers, flags)
    return _CACHED[key]


def _fp8(x, scale):
    return np.clip(x * scale, -240.0, 240.0).astype(E4)


def kernel(X, tok_w, tok_b, pos_w, pos_b, seg_w, seg_b,
           Wq, bq, Wk, bk, Wv, bv, Wo, bo,
           ln1_g, ln1_b, ffp_w, ffp_b, ln2_g, ln2_b, n_layers=L):
    global _LAST_RES
    f32 = np.float32
    X = np.asarray(X, dtype=np.int32)
    tok_w = np.asarray(tok_w, f32); pos_w = np.asarray(pos_w, f32); seg_w = np.asarray(seg_w, f32)
    Wq = np.asarray(Wq, f32); Wk = np.asarray(Wk, f32); Wv = np.asarray(Wv, f32)
    Wo = np.asarray(Wo, f32); ffp_w = np.asarray(ffp_w, f32)
    bq = np.asarray(bq, f32); bk = np.asarray(bk, f32); bv = np.asarray(bv, f32)
    bo = np.asarray(bo, f32); ffp_b = np.asarray(ffp_b, f32)
    ln1_g = np.asarray(ln1_g, f32); ln1_b = np.asarray(ln1_b, f32)
    ln2_g = np.asarray(ln2_g, f32); ln2_b = np.asarray(ln2_b, f32)
    tok_b = np.asarray(tok_b, f32); pos_b = np.asarray(pos_b, f32); seg_b = np.asarray(seg_b, f32)

    emb_bias = tok_b + pos_b + seg_b
    flags = {
        "emb_bias": bool(np.any(emb_bias)),
        "ln1": bool(np.any(ln1_g != 1) or np.any(ln1_b)),
        "ln2": bool(np.any(ln2_g != 1) or np.any(ln2_b)),
        "mask": bool(np.any(X[:, 0, :] == 0)),
    }
    assert not (np.any(bo) or np.any(ffp_b) or np.any(bq) or np.any(bk) or np.any(bv)), \
        "nonzero attention/ffn biases not implemented in this specialization"
    flag_key = tuple(sorted(flags.items()))
    nc = _get_nc(n_layers, flag_key, flags)

    tok_wb = tok_w.astype(BF)
    pos_wb = pos_w.astype(BF)
    seg_wb = seg_w.astype(BF)

    in_maps = []
    per_g = {}
    nl = n_layers
    for g in range(2):
        hsl = slice(g * HPC, (g + 1) * HPC)
        # [L, D, HK] per-group flattened weights
        wq_f = np.ascontiguousarray(Wq[:nl, :, hsl, :]).reshape(nl, D, HK)
        wk_f = np.ascontiguousarray(Wk[:nl, :, hsl, :]).reshape(nl, D, HK)
        wv_f = np.ascontiguousarray(Wv[:nl, :, hsl, :]).reshape(nl, D, HK)
        wo_f = np.ascontiguousarray(Wo[:nl, hsl, :, :]).reshape(nl, HK, D)
        # fp8 pair layout [L, h, p, j, i, 768]
        wq8 = np.ascontiguousarray(
            _fp8(wq_f, W_SC).reshape(nl, JD, 2, 128, HPC, KH).transpose(0, 4, 3, 1, 2, 5))
        wk8 = np.ascontiguousarray(
            _fp8(wk_f, W_SC).reshape(nl, JD, 2, 128, HPC, KH).transpose(0, 4, 3, 1, 2, 5))
        # bf16 per-head layouts
        wvh = np.ascontiguousarray(
            wv_f.astype(BF).reshape(nl, DC, 128, HPC, KH).transpose(0, 3, 2, 1, 4))
        woh = np.ascontiguousarray(
            wo_f.astype(BF).reshape(nl, HPC, DC, 128, D).transpose(0, 1, 3, 2, 4))
        per_g[g] = {"wq8": wq8, "wk8": wk8, "wv": wvh, "wo": woh}
    ffh = np.ascontiguousarray(
        ffp_w[:nl].astype(BF).reshape(nl, DC, 128, D).transpose(0, 2, 1, 3))

    for c in range(NCORES):
        b, g = c // 2, c % 2
        m = {
            "xids": np.ascontiguousarray(X[b]),
            "tok_w": tok_wb, "pos_w": pos_wb, "seg_w": seg_wb,
            "ff": ffh,
            **per_g[g],
        }
        if flags["emb_bias"]:
            m["emb_bias"] = emb_bias
        if flags["ln1"]:
            m["ln1_g"] = np.ascontiguousarray(ln1_g[:nl])
            m["ln1_b"] = np.ascontiguousarray(ln1_b[:nl])
        if flags["ln2"]:
            m["ln2_g"] = np.ascontiguousarray(ln2_g[:nl])
            m["ln2_b"] = np.ascontiguousarray(ln2_b[:nl])
        if flags["mask"]:
            m["maskneg"] = np.where(X[b, 0, :] == 0, -1e9 * QK_SC * QK_SC, 0.0).astype(f32)
        in_maps.append(m)

    res = bass_utils.run_bass_kernel_spmd(nc, in_maps, core_ids=list(range(NCORES)))
    _LAST_RES = res
    out = np.empty((B, S, D), np.float32)
    for b in range(B):
        o0 = res.results[2 * b]["out"]      # rank-0 shards: rows 0:128 / 256:384
        o1 = res.results[2 * b + 1]["out"]  # rank-1 shards: rows 128:256 / 384:512
        out[b, 0:128] = o0[0:128]
        out[b, 128:256] = o1[0:128]
        out[b, 256:384] = o0[128:256]
        out[b, 384:512] = o1[128:256]
    return out


# revision 4
# speedup vs baseline: 1.0097x; 1.0097x over previous
"""Trainium2 Bass kernel for nn_JslBERT — v2: fp8 DoubleRow Q/K/scores.

Sharding: 8 cores = 4 batch x 2 head-groups (6 heads each). Per layer,
attention-output partials are pairwise AllReduced in bf16; LN+FFN run
redundantly on both cores of a pair.

v2 changes vs baseline:
 - Q-proj, K-proj and QK^T scores run in fp8(e4m3) with perf_mode=DoubleRow
   (contraction pairs of 128-chunks per instruction). Static power-of-2
   scaling: weights x1024, residual x16 (x512 in layer 0), Q^T/K^T stored
   x32; descale folded into the PSUM drains and the exp() scale.
   V/ctx/out-proj/FFN stay bf16 (precision-critical path; fp8 there blows
   the 2e-2 gate).
 - Weight DMAs consolidated: one DMA per (tensor, head[, chunk]) instead of
   one per 128-row d-chunk (HWDGE instruction-count was near saturation).
"""
import os
import numpy as np
import ml_dtypes

import concourse.bass as bass
import concourse.bacc as bacc
import concourse.tile as tile
import concourse.bass_utils as bass_utils
from concourse import mybir
from concourse.masks import make_identity

# Model dims (hardcoded per problem spec)
B, S, L, D, H, V, PMAX = 4, 512, 4, 768, 12, 32000, 512
EPS = 1e-3
NCORES = 8
HPC = H // 2          # heads per core
KH = D                # head dim (768)
HK = HPC * KH         # 4608 flattened head dims per core
SCALE = 1.0 / float(np.sqrt(D))

F32 = mybir.dt.float32
BF16 = mybir.dt.bfloat16
F8 = mybir.dt.float8e4
I32 = mybir.dt.int32
DR = mybir.MatmulPerfMode.DoubleRow

TT = S // 128         # 4 t-tiles total
DC = D // 128         # 6 d-chunks
JD = DC // 2          # 3 d-chunk pairs (DoubleRow)
NCH = [(0, 512), (512, 256)]  # free-dim chunks for width-768 outputs
NTC = 2               # t-chunks per sequence
TPC = TT // NTC       # 128-tiles per chunk (2)
CW = S // NTC         # chunk width (256)

# fp8 static scales
W_SC = 1024.0                     # wq/wk weights
QK_SC = 32.0                      # stored Q^T / K^T
EXP_SCALE = SCALE / (QK_SC * QK_SC)


def a_sc(li):                     # residual-stream fp8 scale
    return 512.0 if li == 0 else 16.0


def qk_drain(li):                 # PSUM -> qt/kt fp8 multiplier
    return QK_SC / (a_sc(li) * W_SC)


BF = np.dtype(ml_dtypes.bfloat16)
E4 = np.dtype(ml_dtypes.float8_e4m3)


def build_nc(n_layers=L, flags=None):
    """Build the Bass graph. flags: dict of which optional inputs exist."""
    flags = flags or {}
    nc = bacc.Bacc("TRN2", target_bir_lowering=False, debug=False,
                   num_devices=NCORES)

    xids_d = nc.dram_tensor("xids", [3, S], I32, kind="ExternalInput").ap()
    tokw_d = nc.dram_tensor("tok_w", [V, D], BF16, kind="ExternalInput").ap()
    posw_d = nc.dram_tensor("pos_w", [PMAX, D], BF16, kind="ExternalInput").ap()
    segw_d = nc.dram_tensor("seg_w", [2, D], BF16, kind="ExternalInput").ap()
    # fp8 pair layouts: [L, h, p, j, i, 768] with slot i = d-chunk 2j+i
    wq_d = nc.dram_tensor("wq8", [n_layers, HPC, 128, JD, 2, KH], F8, kind="ExternalInput").ap()
    wk_d = nc.dram_tensor("wk8", [n_layers, HPC, 128, JD, 2, KH], F8, kind="ExternalInput").ap()
    # bf16 per-head layouts: [L, h, p, dc|kc, 768]
    wv_d = nc.dram_tensor("wv", [n_layers, HPC, 128, DC, KH], BF16, kind="ExternalInput").ap()
    wo_d = nc.dram_tensor("wo", [n_layers, HPC, 128, DC, D], BF16, kind="ExternalInput").ap()
    ff_d = nc.dram_tensor("ff", [n_layers, 128, DC, D], BF16, kind="ExternalInput").ap()
    out_d = nc.dram_tensor("out", [S, D], F32, kind="ExternalOutput").ap()

    opt = {}
    if flags.get("emb_bias"):
        opt["emb_bias"] = nc.dram_tensor("emb_bias", [D], F32, kind="ExternalInput").ap()
    for nm in ("ln1", "ln2"):
        if flags.get(nm):
            opt[nm + "_g"] = nc.dram_tensor(nm + "_g", [n_layers, D], F32, kind="ExternalInput").ap()
            opt[nm + "_b"] = nc.dram_tensor(nm + "_b", [n_layers, D], F32, kind="ExternalInput").ap()
    if flags.get("mask"):
        opt["maskneg"] = nc.dram_tensor("maskneg", [S], F32, kind="ExternalInput").ap()

    with tile.TileContext(nc) as tc:
        import contextlib
        with contextlib.ExitStack() as ctx:
            _build_body(ctx, tc, n_layers, flags, xids_d, tokw_d, posw_d, segw_d,
                        wq_d, wk_d, wv_d, wo_d, ff_d, out_d, opt)
    nc.compile()
    return nc


def _build_body(ctx, tc, n_layers, flags, xids_d, tokw_d, posw_d, segw_d,
                wq_d, wk_d, wv_d, wo_d, ff_d, out_d, opt):
    nc = tc.nc

    const = ctx.enter_context(tc.tile_pool(name="const", bufs=1))
    wq8_pool = ctx.enter_context(tc.tile_pool(name="wq8", bufs=3))
    wk8_pool = ctx.enter_context(tc.tile_pool(name="wk8", bufs=3))
    wbig_pool = ctx.enter_context(tc.tile_pool(name="wbig", bufs=5))   # wv/wo
    ff_pool = ctx.enter_context(tc.tile_pool(name="ffp", bufs=1))
    rT8_pool = ctx.enter_context(tc.tile_pool(name="rT8", bufs=8))
    rT_pool = ctx.enter_context(tc.tile_pool(name="rT", bufs=13))
    kt_pool = ctx.enter_context(tc.tile_pool(name="ktp", bufs=18))
    v_pool = ctx.enter_context(tc.tile_pool(name="vp", bufs=24))
    qt_pool = ctx.enter_context(tc.tile_pool(name="qtp", bufs=19))
    pe_pool = ctx.enter_context(tc.tile_pool(name="pep", bufs=5))
    pt_pool = ctx.enter_context(tc.tile_pool(name="ptp", bufs=8))
    ct_pool = ctx.enter_context(tc.tile_pool(name="ctp", bufs=5))
    xtd_pool = ctx.enter_context(tc.tile_pool(name="xtd", bufs=6))
    accb_pool = ctx.enter_context(tc.tile_pool(name="accb", bufs=4))
    sm_pool = ctx.enter_context(tc.tile_pool(name="sm", bufs=8))
    ps_mm = ctx.enter_context(tc.tile_pool(name="psmm", bufs=6, space="PSUM"))
    ps_tp = ctx.enter_context(tc.tile_pool(name="pstp", bufs=2, space="PSUM"))
    dram = ctx.enter_context(tc.tile_pool(name="dram", bufs=2, space="DRAM"))

    ident = const.tile([128, 128], F32)
    make_identity(nc, ident[:])
    identb = const.tile([128, 128], BF16)
    make_identity(nc, identb[:])
    eps_t = const.tile([128, 1], F32)
    nc.vector.memset(eps_t[:], EPS)
    ones_t = const.tile([128, 1], BF16)
    nc.vector.memset(ones_t[:], 1.0)

    def mm_tile():
        return ps_mm.tile([128, 512], F32, tag="mm", name="mmps")

    # ---- weight loaders ------------------------------------------------
    def load_wq8(li, h):
        t = wq8_pool.tile([128, JD, 2, KH], F8, tag="wq8")
        nc.sync.dma_start(t[:], wq_d[li, h])
        return t

    def load_wk8(li, h):
        t = wk8_pool.tile([128, JD, 2, KH], F8, tag="wk8")
        nc.sync.dma_start(t[:], wk_d[li, h])
        return t

    def load_big(wd, li, h=None):
        pool = wbig_pool if h is not None else ff_pool
        t = pool.tile([128, DC, D], BF16, tag="wbig" if h is not None else "ff")
        nc.sync.dma_start(t[:], wd[li] if h is None else wd[li, h])
        return t

    # ---- embeddings ----------------------------------------------------
    idx = const.tile([128, 3, TT], I32)
    nc.sync.dma_start(idx[:], xids_d.rearrange("k (j p) -> p k j", p=128))

    emb_bias_ap = None
    if "emb_bias" in opt:
        eb = const.tile([128, DC], F32)
        nc.sync.dma_start(eb[:], opt["emb_bias"].rearrange("(c p) -> p c", p=128))
        emb_bias_ap = [eb[:, c:c + 1] for c in range(DC)]

    # pos ids are structurally arange(S) (built that way in the model), so the
    # pos "lookup" is a direct row DMA; tok/seg stay data-dependent gathers.
    tok_t = wbig_pool.tile([128, DC, D], BF16, tag="wbig", name="tokt")
    pos_t = wbig_pool.tile([128, DC, D], BF16, tag="wbig", name="post")
    seg_t = wbig_pool.tile([128, DC, D], BF16, tag="wbig", name="segt")
    nc.sync.dma_start(pos_t[:, 0:TT, :],
                      posw_d.rearrange("(a p) d -> p a d", p=128))
    for tm in range(TT):
        nc.gpsimd.indirect_dma_start(
            out=tok_t[:, tm, :], out_offset=None, in_=tokw_d[:],
            in_offset=bass.IndirectOffsetOnAxis(ap=idx[:, 0, tm:tm + 1], axis=0))
        nc.gpsimd.indirect_dma_start(
            out=seg_t[:, tm, :], out_offset=None, in_=segw_d[:],
            in_offset=bass.IndirectOffsetOnAxis(ap=idx[:, 2, tm:tm + 1], axis=0))
    for tm in range(TT):
        nc.vector.tensor_add(tok_t[:, tm, :], tok_t[:, tm, :], pos_t[:, tm, :])
        nc.vector.tensor_add(tok_t[:, tm, :], tok_t[:, tm, :], seg_t[:, tm, :])

    # resTc[tcix][dc]: [128 d, 256 t] bf16 (V-proj stationary)
    # rT8c[tcix][j]:  [128 d, 2, 256 t] fp8 pairs (Q/K moving operand)
    resTc = [[None] * DC for _ in range(NTC)]
    rT8c = [[None] * JD for _ in range(NTC)]
    for tcix in range(NTC):
        for dc in range(DC):
            pp = ps_tp.tile([128, CW], BF16, tag="tp", name="tpps")
            for tl in range(TPC):
                nc.tensor.transpose(pp[:, tl * 128:(tl + 1) * 128],
                                    tok_t[:, tcix * TPC + tl, dc * 128:(dc + 1) * 128],
                                    identb[:])
            rt = rT_pool.tile([128, CW], BF16, tag="rT")
            if emb_bias_ap is not None:
                nc.vector.tensor_scalar_add(rt[:], pp[:], emb_bias_ap[dc])
            else:
                nc.vector.tensor_copy(out=rt[:], in_=pp[:])
            resTc[tcix][dc] = rt
            if dc % 2 == 0:
                rT8c[tcix][dc // 2] = rT8_pool.tile([128, 2, CW], F8, tag="rT8", name="rt8e")
            if emb_bias_ap is not None:
                nc.vector.tensor_scalar(out=rT8c[tcix][dc // 2][:, dc % 2, :],
                                        in0=pp[:], scalar1=emb_bias_ap[dc],
                                        scalar2=a_sc(0),
                                        op0=mybir.AluOpType.add,
                                        op1=mybir.AluOpType.mult)
            else:
                nc.vector.tensor_scalar_mul(rT8c[tcix][dc // 2][:, dc % 2, :],
                                            pp[:], a_sc(0))

    mask_ap = None
    if "maskneg" in opt:
        mk = const.tile([128, TT], F32)
        nc.sync.dma_start(mk[:], opt["maskneg"].rearrange("(a p) -> p a", p=128))
        mask_ap = mk

    # ---- per-layer helpers --------------------------------------------
    QT_AHEAD = 6  # heads whose chunk-0 QT is prefetched at the end of the previous layer

    def load_ln_gb(li, nm):
        if nm + "_g" not in opt:
            return None
        gb = const.tile([128, 2, D], F32, tag=f"lngb{nm}{li}")
        nc.sync.dma_start(gb[:, 0, :], opt[nm + "_g"][li].partition_broadcast(128))
        nc.sync.dma_start(gb[:, 1, :], opt[nm + "_b"][li].partition_broadcast(128))
        return gb

    def layernorm(aps, gb):
        for x in aps:
            stats = sm_pool.tile([128, 3, 6], F32, tag="bnst")
            mv = sm_pool.tile([128, 2], F32, tag="bnmv")
            xg = x.rearrange("p (a c) -> p a c", a=3)
            for a in range(3):
                nc.vector.bn_stats(out=stats[:, a, :], in_=xg[:, a, :])
            nc.vector.bn_aggr(out=mv[:], in_=stats[:])
            rstd = sm_pool.tile([128, 1], F32, tag="rstd")
            nc.scalar.activation(out=rstd[:], in_=mv[:, 1:2],
                                 func=mybir.ActivationFunctionType.Sqrt,
                                 bias=eps_t[:], scale=1.0)
            nc.vector.reciprocal(rstd[:], rstd[:])
            nc.vector.tensor_scalar(out=x, in0=x, scalar1=mv[:, 0:1],
                                    scalar2=rstd[:],
                                    op0=mybir.AluOpType.subtract,
                                    op1=mybir.AluOpType.mult)
            if gb is not None:
                nc.vector.tensor_mul(x, x, gb[:, 0, :])
                nc.vector.tensor_add(x, x, gb[:, 1, :])

    def emit_kv_half(li, sc, h, rT8_l, resTc_l, kt_all, v_all):
        """KT s-half (fp8 DoubleRow) + V s-half (bf16) for one head.
        kt[(h,j)]: [128 k, 2, 512 s] fp8 pairs, v[(h,sm)]: [128 s, 768 k] bf16."""
        wk8 = load_wk8(li, h)
        wv_t = load_big(wv_d, li, h)
        dr_sc = qk_drain(li)
        for m2 in range(JD):
            pm = mm_tile()
            for half in range(2):
                m = 2 * m2 + half
                for j in range(JD):
                    nc.tensor.matmul(pm[:, half * CW:half * CW + CW],
                                     wk8[:, j, :, m * 128:(m + 1) * 128],
                                     rT8_l[sc][j][:],
                                     start=(j == 0), stop=(j == JD - 1),
                                     perf_mode=DR)
            if sc == 0:
                kt_all[(h, m2)] = kt_pool.tile([128, 2, S], F8, tag="kt",
                                               name=f"kt{h}_{m2}")
            pmv = pm[:].rearrange("p (i c) -> p i c", i=2)
            if m2 % 2 == 0:
                nc.scalar.mul(kt_all[(h, m2)][:, :, sc * CW:(sc + 1) * CW],
                              pmv, dr_sc)
            else:
                nc.vector.tensor_scalar_mul(
                    kt_all[(h, m2)][:, :, sc * CW:(sc + 1) * CW],
                    pmv, dr_sc)
        for tl in range(TPC):
            sm = sc * TPC + tl
            vt = v_pool.tile([128, D], BF16, tag="v")
            v_all[(h, sm)] = vt
            for (n0, nw) in NCH:
                pm = mm_tile()
                for dc in range(DC):
                    nc.tensor.matmul(pm[:, :nw],
                                     resTc_l[sc][dc][:, tl * 128:(tl + 1) * 128],
                                     wv_t[:, dc, n0:n0 + nw],
                                     start=(dc == 0), stop=(dc == DC - 1))
                nc.scalar.copy(out=vt[:, n0:n0 + nw], in_=pm[:, :nw])

    def emit_qt(li, tcix, h, rT8_l):
        """QT for one head/chunk via fp8 DoubleRow, packed 2 m's per PSUM bank.
        Returns 3 tiles [128, 512] fp8: tile j = m (2j, 2j+1) x 256 t."""
        wq8 = load_wq8(li, h)
        dr_sc = qk_drain(li)
        qt_sb = []
        for jo in range(JD):
            pm = mm_tile()
            for half in range(2):
                m = 2 * jo + half
                for j in range(JD):
                    nc.tensor.matmul(pm[:, half * CW:half * CW + CW],
                                     wq8[:, j, :, m * 128:(m + 1) * 128],
                                     rT8_l[tcix][j][:],
                                     start=(j == 0), stop=(j == JD - 1),
                                     perf_mode=DR)
            ot = qt_pool.tile([128, 512], F8, tag="qt")
            if jo % 2 == 0:
                nc.vector.tensor_scalar_mul(ot[:], pm[:], dr_sc)
            else:
                nc.scalar.mul(ot[:], pm[:], dr_sc)
            qt_sb.append(ot)
        return qt_sb

    def emit_scores(li, tcix, h, qt_sb, kt_all):
        """Transposed scores: peT[s, t] = exp(K Q^T) via fp8 DoubleRow, plus
        per-t 1/sum (applied later at the out-proj drain). Removes the whole
        P-transpose stage. pe_pair[smp] is [128 s, (2 sm-half)(256 t)] bf16 —
        the same layout the old pt_sb had, so ctx consumes it unchanged."""
        pe_pair = []
        for smp in range(2):
            pm = mm_tile()
            for half in range(2):
                sm = 2 * smp + half
                for j in range(JD):
                    qv = qt_sb[j][:].rearrange("p (i c) -> p i c", i=2)
                    nc.tensor.matmul(pm[:, half * CW:half * CW + CW],
                                     kt_all[(h, j)][:, :, sm * 128:(sm + 1) * 128],
                                     qv,
                                     start=(j == 0), stop=(j == JD - 1),
                                     perf_mode=DR)
                if mask_ap is not None:
                    nc.vector.tensor_scalar_add(pm[:, half * CW:half * CW + CW],
                                                pm[:, half * CW:half * CW + CW],
                                                mask_ap[:, sm:sm + 1])
            pe = pe_pool.tile([128, S], BF16, tag="pe")
            nc.scalar.activation(out=pe[:], in_=pm[:],
                                 func=mybir.ActivationFunctionType.Exp,
                                 scale=EXP_SCALE)
            pe_pair.append(pe)
        sp = ps_tp.tile([128, 2], F32, tag="tp", name="sumsps")
        for tb in range(TPC):
            k = 0
            for smp in range(2):
                for half in range(2):
                    nc.tensor.matmul(sp[:, tb:tb + 1],
                                     pe_pair[smp][:, half * CW + tb * 128:
                                                  half * CW + (tb + 1) * 128],
                                     ones_t[:], start=(k == 0), stop=(k == 3))
                    k += 1
        rec = sm_pool.tile([128, 2], F32, tag="rec")
        nc.vector.reciprocal(rec[:], sp[:])
        return pe_pair, rec

    def emit_ptco(li, tcix, h, pe_pair, rec, v_all, acc, accb):
        """ctxT from unnormalized peT, out-proj partial with 1/sum folded
        into the accumulate as a per-t-partition scalar."""
        # ctxT packed: tile j holds km (2j | 2j+1) x 256 t (unnormalized)
        ct_sb = []
        for j in range(DC // 2):
            pm = mm_tile()
            for half in range(2):
                km = 2 * j + half
                for sm in range(TT):
                    nc.tensor.matmul(pm[:, half * CW:half * CW + CW],
                                     v_all[(h, sm)][:, km * 128:(km + 1) * 128],
                                     pe_pair[sm // 2][:, (sm % 2) * CW:(sm % 2) * CW + CW],
                                     start=(sm == 0), stop=(sm == TT - 1))
            ot = ct_pool.tile([128, 512], BF16, tag="ct")
            nc.vector.tensor_copy(out=ot[:], in_=pm[:])
            ct_sb.append(ot)

        wo_t = load_big(wo_d, li, h)
        for tl in range(TPC):
            for (n0, nw) in NCH:
                pm = mm_tile()
                for kc in range(DC):
                    nc.tensor.matmul(pm[:, :nw],
                                     ct_sb[kc // 2][:, (kc % 2) * CW + tl * 128:
                                                    (kc % 2) * CW + (tl + 1) * 128],
                                     wo_t[:, kc, n0:n0 + nw],
                                     start=(kc == 0), stop=(kc == DC - 1))
                if h == 0:
                    nc.vector.tensor_scalar_mul(acc[tl][:, n0:n0 + nw],
                                                pm[:, :nw], rec[:, tl:tl + 1])
                elif h < HPC - 1:
                    nc.vector.scalar_tensor_tensor(
                        out=acc[tl][:, n0:n0 + nw], in0=pm[:, :nw],
                        scalar=rec[:, tl:tl + 1], in1=acc[tl][:, n0:n0 + nw],
                        op0=mybir.AluOpType.mult, op1=mybir.AluOpType.add)
                else:
                    nc.vector.scalar_tensor_tensor(
                        out=accb[:, tl, n0:n0 + nw], in0=pm[:, :nw],
                        scalar=rec[:, tl:tl + 1], in1=acc[tl][:, n0:n0 + nw],
                        op0=mybir.AluOpType.mult, op1=mybir.AluOpType.add)

    def emit_collective(li, accb):
        arin = dram.tile([CW, D], BF16, tag="arin")
        last = li == n_layers - 1
        nc.sync.dma_start(arin[:].rearrange("(a p) d -> p a d", p=128),
                          accb[:, 0:TPC, :])
        if last:
            arout = dram.tile([128, D], BF16, tag="arout2")
            nc.gpsimd.collective_compute(
                "ReduceScatter", mybir.AluOpType.add,
                replica_groups=[[0, 1], [2, 3], [4, 5], [6, 7]],
                ins=[arin.opt()], outs=[arout.opt()])
        else:
            # AllGather both partials (no AllReduce cost multiplier on the
            # collective cores); the pairwise add happens locally on DVE.
            arout = dram.tile([2 * CW, D], BF16, tag="arout")
            nc.gpsimd.collective_compute(
                "AllGather", mybir.AluOpType.bypass,
                replica_groups=[[0, 1], [2, 3], [4, 5], [6, 7]],
                ins=[arin.opt()], outs=[arout.opt()])
        return arout

    def emit_tail_chunk(li, tcix, arout, gb1, gb2, ff_t, resTc_next, rT8_next):
        """AR result -> LN1 -> FFN -> LN2 -> resTc_next[tcix] (or output DMA).
        For the last layer the collective was a ReduceScatter: each core owns
        128 of the 256 chunk rows; the host reassembles."""
        ntl = 1 if li == n_layers - 1 else TPC
        xc = accb_pool.tile([128, TPC, D], BF16, tag="accb", name="xcur")
        if li == n_layers - 1:
            nc.gpsimd.dma_start(xc[:, 0:ntl, :],
                                arout[:].rearrange("(a p) d -> p a d", p=128))
        else:
            xg = wbig_pool.tile([128, 2, TPC, D], BF16, tag="wbig", name="xg")
            nc.gpsimd.dma_start(xg[:],
                                arout[:].rearrange("(g a p) d -> p g a d", p=128, g=2))
            nc.vector.tensor_add(xc[:, 0:ntl, :], xg[:, 0, :, :], xg[:, 1, :, :])
        xcur = [xc[:, tl, :] for tl in range(ntl)]
        layernorm(xcur, gb1)

        lnT = []
        for dc in range(DC):
            pp = ps_tp.tile([128, CW], BF16, tag="tp", name="tpps")
            for tl in range(ntl):
                nc.tensor.transpose(pp[:, tl * 128:(tl + 1) * 128],
                                    xc[:, tl, dc * 128:(dc + 1) * 128],
                                    identb[:])
            t = pt_pool.tile([128, 512], BF16, tag="pts", name="lnT")
            nc.scalar.copy(out=t[:, :ntl * 128], in_=pp[:, :ntl * 128])
            lnT.append(t)

        xmid = [xtd_pool.tile([128, D], F32, tag="xtd", name=f"xmid{tl}") for tl in range(ntl)]
        xmid_ap = [t[:] for t in xmid]
        for tl in range(ntl):
            for (n0, nw) in NCH:
                pm = mm_tile()
                for dc in range(DC):
                    nc.tensor.matmul(pm[:, :nw], lnT[dc][:, tl * 128:(tl + 1) * 128],
                                     ff_t[:, dc, n0:n0 + nw],
                                     start=(dc == 0), stop=(dc == DC - 1))
                nc.vector.tensor_copy(out=xmid[tl][:, n0:n0 + nw], in_=pm[:, :nw])

        layernorm(xmid_ap, gb2)

        if li < n_layers - 1:
            for dc in range(DC):
                pp = ps_tp.tile([128, CW], F32, tag="tp", name="tpps")
                for tl in range(TPC):
                    nc.tensor.transpose(pp[:, tl * 128:(tl + 1) * 128],
                                        xmid[tl][:, dc * 128:(dc + 1) * 128],
                                        ident[:])
                rt = rT_pool.tile([128, CW], BF16, tag="rT")
                nc.scalar.copy(out=rt[:], in_=pp[:])
                resTc_next[tcix][dc] = rt
                if dc % 2 == 0:
                    rT8_next[tcix][dc // 2] = rT8_pool.tile([128, 2, CW], F8, tag="rT8", name="rt8n")
                nc.vector.tensor_scalar_mul(rT8_next[tcix][dc // 2][:, dc % 2, :],
                                            pp[:], a_sc(li + 1))
        else:
            nc.sync.dma_start(out_d[tcix * 128:(tcix + 1) * 128, :], xmid[0][:])

    # ---- layers --------------------------------------------------------
    # prologue: layer-0 chunk-0 KV + QT prefetch (resTc from embeddings)
    cur_kt, cur_v = {}, {}
    qt_pre = {}
    for h in range(HPC):
        emit_kv_half(0, 0, h, rT8c, resTc, cur_kt, cur_v)
        if h < QT_AHEAD:
            qt_pre[h] = emit_qt(0, 0, h, rT8c)

    for li in range(n_layers):
        gb1 = load_ln_gb(li, "ln1")
        gb2 = load_ln_gb(li, "ln2")

        # A: KV s-half 1 (skewed) + chunk-0 score chains
        acc0 = [xtd_pool.tile([128, D], F32, tag="xtd", name=f"acc{tl}") for tl in range(TPC)]
        accb0 = accb_pool.tile([128, TPC, D], BF16, tag="accb", name="accb0")
        emit_kv_half(li, 1, 0, rT8c, resTc, cur_kt, cur_v)
        pend = None
        for h in range(HPC):
            if h + 1 < HPC:
                emit_kv_half(li, 1, h + 1, rT8c, resTc, cur_kt, cur_v)
            qt_sb = qt_pre.pop(h) if h in qt_pre else emit_qt(li, 0, h, rT8c)
            sc_out = emit_scores(li, 0, h, qt_sb, cur_kt)
            if pend is not None:
                emit_ptco(li, 0, pend[0], pend[1], pend[2], cur_v, acc0, accb0)
            pend = (h,) + sc_out
        emit_ptco(li, 0, pend[0], pend[1], pend[2], cur_v, acc0, accb0)
        arout0 = emit_collective(li, accb0)

        # C: chunk-1 score chains (AR(c0) overlaps this)
        acc1 = [xtd_pool.tile([128, D], F32, tag="xtd", name=f"acc{tl}") for tl in range(TPC)]
        accb1 = accb_pool.tile([128, TPC, D], BF16, tag="accb", name="accb1")
        pend = None
        for h in range(HPC):
            qt_sb = emit_qt(li, 1, h, rT8c)
            sc_out = emit_scores(li, 1, h, qt_sb, cur_kt)
            if pend is not None:
                emit_ptco(li, 1, pend[0], pend[1], pend[2], cur_v, acc1, accb1)
            pend = (h,) + sc_out
        emit_ptco(li, 1, pend[0], pend[1], pend[2], cur_v, acc1, accb1)

        ff_t = load_big(ff_d, li)

        # E: tail chunk 0 — emitted before the chunk-1 collective so its
        # gathered-load/add/LN serial chain runs during C on the free engines
        resTc_next = [[None] * DC for _ in range(NTC)]
        rT8_next = [[None] * JD for _ in range(NTC)]
        emit_tail_chunk(li, 0, arout0, gb1, gb2, ff_t, resTc_next, rT8_next)
        arout1 = emit_collective(li, accb1)

        # F: next layer's chunk-0 KV + QT prefetch (fills AR(c1) window)
        next_kt, next_v = {}, {}
        qt_pre = {}
        if li < n_layers - 1:
            for h in range(HPC):
                emit_kv_half(li + 1, 0, h, rT8_next, resTc_next, next_kt, next_v)
                if h < QT_AHEAD:
                    qt_pre[h] = emit_qt(li + 1, 0, h, rT8_next)

        # G: tail chunk 1
        emit_tail_chunk(li, 1, arout1, gb1, gb2, ff_t, resTc_next, rT8_next)

        resTc = resTc_next
        rT8c = rT8_next
        cur_kt, cur_v = next_kt, next_v


# ------------------------------------------------------------------------
# host side
# ------------------------------------------------------------------------
_CACHED = {}
_LAST_RES = None


def _get_nc(n_layers, flag_key, flags):
    key = (n_layers, flag_key)
    if key not in _CACHED:
        _CACHED[key] = build_nc(n_layers, flags)
    return _CACHED[key]


def _fp8(x, scale):
    return np.clip(x * scale, -240.0, 240.0).astype(E4)


def kernel(X, tok_w, tok_b, pos_w, pos_b, seg_w, seg_b,
           Wq, bq, Wk, bk, Wv, bv, Wo, bo,
           ln1_g, ln1_b, ffp_w, ffp_b, ln2_g, ln2_b, n_layers=L):
    global _LAST_RES
    f32 = np.float32
    X = np.asarray(X, dtype=np.int32)
    tok_w = np.asarray(tok_w, f32); pos_w = np.asarray(pos_w, f32); seg_w = np.asarray(seg_w, f32)
    Wq = np.asarray(Wq, f32); Wk = np.asarray(Wk, f32); Wv = np.asarray(Wv, f32)
    Wo = np.asarray(Wo, f32); ffp_w = np.asarray(ffp_w, f32)
    bq = np.asarray(bq, f32); bk = np.asarray(bk, f32); bv = np.asarray(bv, f32)
    bo = np.asarray(bo, f32); ffp_b = np.asarray(ffp_b, f32)
    ln1_g = np.asarray(ln1_g, f32); ln1_b = np.asarray(ln1_b, f32)
    ln2_g = np.asarray(ln2_g, f32); ln2_b = np.asarray(ln2_b, f32)
    tok_b = np.asarray(tok_b, f32); pos_b = np.asarray(pos_b, f32); seg_b = np.asarray(seg_b, f32)

    emb_bias = tok_b + pos_b + seg_b
    flags = {
        "emb_bias": bool(np.any(emb_bias)),
        "ln1": bool(np.any(ln1_g != 1) or np.any(ln1_b)),
        "ln2": bool(np.any(ln2_g != 1) or np.any(ln2_b)),
        "mask": bool(np.any(X[:, 0, :] == 0)),
    }
    assert not (np.any(bo) or np.any(ffp_b) or np.any(bq) or np.any(bk) or np.any(bv)), \
        "nonzero attention/ffn biases not implemented in this specialization"
    flag_key = tuple(sorted(flags.items()))
    nc = _get_nc(n_layers, flag_key, flags)

    tok_wb = tok_w.astype(BF)
    pos_wb = pos_w.astype(BF)
    seg_wb = seg_w.astype(BF)

    in_maps = []
    per_g = {}
    nl = n_layers
    for g in range(2):
        hsl = slice(g * HPC, (g + 1) * HPC)
        # [L, D, HK] per-group flattened weights
        wq_f = np.ascontiguousarray(Wq[:nl, :, hsl, :]).reshape(nl, D, HK)
        wk_f = np.ascontiguousarray(Wk[:nl, :, hsl, :]).reshape(nl, D, HK)
        wv_f = np.ascontiguousarray(Wv[:nl, :, hsl, :]).reshape(nl, D, HK)
        wo_f = np.ascontiguousarray(Wo[:nl, hsl, :, :]).reshape(nl, HK, D)
        # fp8 pair layout [L, h, p, j, i, 768]
        wq8 = np.ascontiguousarray(
            _fp8(wq_f, W_SC).reshape(nl, JD, 2, 128, HPC, KH).transpose(0, 4, 3, 1, 2, 5))
        wk8 = np.ascontiguousarray(
            _fp8(wk_f, W_SC).reshape(nl, JD, 2, 128, HPC, KH).transpose(0, 4, 3, 1, 2, 5))
        # bf16 per-head layouts
        wvh = np.ascontiguousarray(
            wv_f.astype(BF).reshape(nl, DC, 128, HPC, KH).transpose(0, 3, 2, 1, 4))
        woh = np.ascontiguousarray(
            wo_f.astype(BF).reshape(nl, HPC, DC, 128, D).transpose(0, 1, 3, 2, 4))
        per_g[g] = {"wq8": wq8, "wk8": wk8, "wv": wvh, "wo": woh}
    ffh = np.ascontiguousarray(
        ffp_w[:nl].astype(BF).reshape(nl, DC, 128, D).transpose(0, 2, 1, 3))

    for c in range(NCORES):
        b, g = c // 2, c % 2
        m = {
            "xids": np.ascontiguousarray(X[b]),
            "tok_w": tok_wb, "pos_w": pos_wb, "seg_w": seg_wb,
            "ff": ffh,
            **per_g[g],
        }
        if flags["emb_bias"]:
            m["emb_bias"] = emb_bias
        if flags["ln1"]:
            m["ln1_g"] = np.ascontiguousarray(ln1_g[:nl])
            m["ln1_b"] = np.ascontiguousarray(ln1_b[:nl])
        if flags["ln2"]:
            m["ln2_g"] = np.ascontiguousarray(ln2_g[:nl])
            m["ln2_b"] = np.ascontiguousarray(ln2_b[:nl])
        if flags["mask"]:
            m["maskneg"] = np.where(X[b, 0, :] == 0, -1e9 * QK_SC * QK_SC, 0.0).astype(f32)
        in_maps.append(m)

    res = bass_utils.run_bass_kernel_spmd(nc, in_maps, core_ids=list(range(NCORES)))
    _LAST_RES = res
    out = np.empty((B, S, D), np.float32)
    for b in range(B):
        o0 = res.results[2 * b]["out"]      # rank-0 shards: rows 0:128 / 256:384
        o1 = res.results[2 * b + 1]["out"]  # rank-1 shards: rows 128:256 / 384:512
        out[b, 0:128] = o0[0:128]
        out[b, 128:256] = o1[0:128]
        out[b, 256:384] = o0[128:256]
        out[b, 384:512] = o1[128:256]
    return out


# revision 5
# speedup vs baseline: 1.0590x; 1.0489x over previous
"""Trainium2 Bass kernel for nn_JslBERT — v2: fp8 DoubleRow Q/K/scores.

Sharding: 8 cores = 4 batch x 2 head-groups (6 heads each). Per layer,
attention-output partials are pairwise AllReduced in bf16; LN+FFN run
redundantly on both cores of a pair.

v2 changes vs baseline:
 - Q-proj, K-proj and QK^T scores run in fp8(e4m3) with perf_mode=DoubleRow
   (contraction pairs of 128-chunks per instruction). Static power-of-2
   scaling: weights x1024, residual x16 (x512 in layer 0), Q^T/K^T stored
   x32; descale folded into the PSUM drains and the exp() scale.
   V/ctx/out-proj/FFN stay bf16 (precision-critical path; fp8 there blows
   the 2e-2 gate).
 - Weight DMAs consolidated: one DMA per (tensor, head[, chunk]) instead of
   one per 128-row d-chunk (HWDGE instruction-count was near saturation).
"""
import os
import numpy as np
import ml_dtypes

import concourse.bass as bass
import concourse.bacc as bacc
import concourse.tile as tile
import concourse.bass_utils as bass_utils
from concourse import mybir
from concourse.masks import make_identity

# Model dims (hardcoded per problem spec)
B, S, L, D, H, V, PMAX = 4, 512, 4, 768, 12, 32000, 512
EPS = 1e-3
NCORES = 8
HPC = H // 2          # heads per core
KH = D                # head dim (768)
HK = HPC * KH         # 4608 flattened head dims per core
SCALE = 1.0 / float(np.sqrt(D))

F32 = mybir.dt.float32
BF16 = mybir.dt.bfloat16
F8 = mybir.dt.float8e4
I32 = mybir.dt.int32
DR = mybir.MatmulPerfMode.DoubleRow

TT = S // 128         # 4 t-tiles total
DC = D // 128         # 6 d-chunks
JD = DC // 2          # 3 d-chunk pairs (DoubleRow)
NCH = [(0, 512), (512, 256)]  # free-dim chunks for width-768 outputs
NTC = 2               # t-chunks per sequence
TPC = TT // NTC       # 128-tiles per chunk (2)
CW = S // NTC         # chunk width (256)

# fp8 static scales
W_SC = 1024.0                     # wq/wk weights
QK_SC = 32.0                      # stored Q^T / K^T
EXP_SCALE = SCALE / (QK_SC * QK_SC)


def a_sc(li):                     # residual-stream fp8 scale
    return 512.0 if li == 0 else 16.0


def qk_drain(li):                 # PSUM -> qt/kt fp8 multiplier
    return QK_SC / (a_sc(li) * W_SC)


BF = np.dtype(ml_dtypes.bfloat16)
E4 = np.dtype(ml_dtypes.float8_e4m3)


def build_nc(n_layers=L, flags=None):
    """Build the Bass graph. flags: dict of which optional inputs exist."""
    flags = flags or {}
    nc = bacc.Bacc("TRN2", target_bir_lowering=False, debug=False,
                   num_devices=NCORES)

    xids_d = nc.dram_tensor("xids", [3, S], I32, kind="ExternalInput").ap()
    tokw_d = nc.dram_tensor("tok_w", [V, D], BF16, kind="ExternalInput").ap()
    posw_d = nc.dram_tensor("pos_w", [PMAX, D], BF16, kind="ExternalInput").ap()
    segw_d = nc.dram_tensor("seg_w", [2, D], BF16, kind="ExternalInput").ap()
    # fp8 pair layouts: [L, h, p, j, i, 768] with slot i = d-chunk 2j+i
    wq_d = nc.dram_tensor("wq8", [n_layers, HPC, 128, JD, 2, KH], F8, kind="ExternalInput").ap()
    wk_d = nc.dram_tensor("wk8", [n_layers, HPC, 128, JD, 2, KH], F8, kind="ExternalInput").ap()
    # bf16 per-head layouts: [L, h, p, dc|kc, 768]
    wv_d = nc.dram_tensor("wv", [n_layers, HPC, 128, DC, KH], BF16, kind="ExternalInput").ap()
    wo_d = nc.dram_tensor("wo", [n_layers, HPC, 128, DC, D], BF16, kind="ExternalInput").ap()
    ff_d = nc.dram_tensor("ff", [n_layers, 128, DC, D], BF16, kind="ExternalInput").ap()
    out_d = nc.dram_tensor("out", [S, D], F32, kind="ExternalOutput").ap()

    opt = {}
    if flags.get("emb_bias"):
        opt["emb_bias"] = nc.dram_tensor("emb_bias", [D], F32, kind="ExternalInput").ap()
    for nm in ("ln1", "ln2"):
        if flags.get(nm):
            opt[nm + "_g"] = nc.dram_tensor(nm + "_g", [n_layers, D], F32, kind="ExternalInput").ap()
            opt[nm + "_b"] = nc.dram_tensor(nm + "_b", [n_layers, D], F32, kind="ExternalInput").ap()
    if flags.get("mask"):
        opt["maskneg"] = nc.dram_tensor("maskneg", [S], F32, kind="ExternalInput").ap()

    with tile.TileContext(nc) as tc:
        import contextlib
        with contextlib.ExitStack() as ctx:
            _build_body(ctx, tc, n_layers, flags, xids_d, tokw_d, posw_d, segw_d,
                        wq_d, wk_d, wv_d, wo_d, ff_d, out_d, opt)
    nc.compile()
    return nc


def _build_body(ctx, tc, n_layers, flags, xids_d, tokw_d, posw_d, segw_d,
                wq_d, wk_d, wv_d, wo_d, ff_d, out_d, opt):
    nc = tc.nc

    const = ctx.enter_context(tc.tile_pool(name="const", bufs=1))
    wq8_pool = ctx.enter_context(tc.tile_pool(name="wq8", bufs=3))
    wk8_pool = ctx.enter_context(tc.tile_pool(name="wk8", bufs=3))
    wbig_pool = ctx.enter_context(tc.tile_pool(name="wbig", bufs=5))   # wv/wo
    ff_pool = ctx.enter_context(tc.tile_pool(name="ffp", bufs=1))
    rT8_pool = ctx.enter_context(tc.tile_pool(name="rT8", bufs=8))
    rT_pool = ctx.enter_context(tc.tile_pool(name="rT", bufs=13))
    kt_pool = ctx.enter_context(tc.tile_pool(name="ktp", bufs=18))
    v_pool = ctx.enter_context(tc.tile_pool(name="vp", bufs=24))
    qt_pool = ctx.enter_context(tc.tile_pool(name="qtp", bufs=19))
    pe_pool = ctx.enter_context(tc.tile_pool(name="pep", bufs=5))
    pt_pool = ctx.enter_context(tc.tile_pool(name="ptp", bufs=8))
    ct_pool = ctx.enter_context(tc.tile_pool(name="ctp", bufs=5))
    xtd_pool = ctx.enter_context(tc.tile_pool(name="xtd", bufs=6))
    accb_pool = ctx.enter_context(tc.tile_pool(name="accb", bufs=4))
    sm_pool = ctx.enter_context(tc.tile_pool(name="sm", bufs=8))
    ps_mm = ctx.enter_context(tc.tile_pool(name="psmm", bufs=6, space="PSUM"))
    ps_tp = ctx.enter_context(tc.tile_pool(name="pstp", bufs=2, space="PSUM"))
    dram = ctx.enter_context(tc.tile_pool(name="dram", bufs=2, space="DRAM"))

    ident = const.tile([128, 128], F32)
    make_identity(nc, ident[:])
    identb = const.tile([128, 128], BF16)
    make_identity(nc, identb[:])
    eps_t = const.tile([128, 1], F32)
    nc.vector.memset(eps_t[:], EPS)
    ones_t = const.tile([128, 1], BF16)
    nc.vector.memset(ones_t[:], 1.0)

    def mm_tile():
        return ps_mm.tile([128, 512], F32, tag="mm", name="mmps")

    # ---- weight loaders ------------------------------------------------
    def load_wq8(li, h):
        t = wq8_pool.tile([128, JD, 2, KH], F8, tag="wq8")
        nc.sync.dma_start(t[:], wq_d[li, h])
        return t

    def load_wk8(li, h):
        t = wk8_pool.tile([128, JD, 2, KH], F8, tag="wk8")
        nc.sync.dma_start(t[:], wk_d[li, h])
        return t

    def load_big(wd, li, h=None):
        pool = wbig_pool if h is not None else ff_pool
        t = pool.tile([128, DC, D], BF16, tag="wbig" if h is not None else "ff")
        nc.sync.dma_start(t[:], wd[li] if h is None else wd[li, h])
        return t

    # ---- embeddings ----------------------------------------------------
    idx = const.tile([128, 3, TT], I32)
    nc.sync.dma_start(idx[:], xids_d.rearrange("k (j p) -> p k j", p=128))

    emb_bias_ap = None
    if "emb_bias" in opt:
        eb = const.tile([128, DC], F32)
        nc.sync.dma_start(eb[:], opt["emb_bias"].rearrange("(c p) -> p c", p=128))
        emb_bias_ap = [eb[:, c:c + 1] for c in range(DC)]

    # pos ids are structurally arange(S) (built that way in the model), so the
    # pos "lookup" is a direct row DMA; tok/seg stay data-dependent gathers.
    tok_t = wbig_pool.tile([128, DC, D], BF16, tag="wbig", name="tokt")
    pos_t = wbig_pool.tile([128, DC, D], BF16, tag="wbig", name="post")
    seg_t = wbig_pool.tile([128, DC, D], BF16, tag="wbig", name="segt")
    nc.sync.dma_start(pos_t[:, 0:TT, :],
                      posw_d.rearrange("(a p) d -> p a d", p=128))
    for tm in range(TT):
        nc.gpsimd.indirect_dma_start(
            out=tok_t[:, tm, :], out_offset=None, in_=tokw_d[:],
            in_offset=bass.IndirectOffsetOnAxis(ap=idx[:, 0, tm:tm + 1], axis=0))
        nc.gpsimd.indirect_dma_start(
            out=seg_t[:, tm, :], out_offset=None, in_=segw_d[:],
            in_offset=bass.IndirectOffsetOnAxis(ap=idx[:, 2, tm:tm + 1], axis=0))
    for tm in range(TT):
        nc.vector.tensor_add(tok_t[:, tm, :], tok_t[:, tm, :], pos_t[:, tm, :])
        nc.vector.tensor_add(tok_t[:, tm, :], tok_t[:, tm, :], seg_t[:, tm, :])

    # resTc[tcix][dc]: [128 d, 256 t] bf16 (V-proj stationary)
    # rT8c[tcix][j]:  [128 d, 2, 256 t] fp8 pairs (Q/K moving operand)
    resTc = [[None] * DC for _ in range(NTC)]
    rT8c = [[None] * JD for _ in range(NTC)]
    for tcix in range(NTC):
        for dc in range(DC):
            pp = ps_tp.tile([128, CW], BF16, tag="tp", name="tpps")
            for tl in range(TPC):
                nc.tensor.transpose(pp[:, tl * 128:(tl + 1) * 128],
                                    tok_t[:, tcix * TPC + tl, dc * 128:(dc + 1) * 128],
                                    identb[:])
            rt = rT_pool.tile([128, CW], BF16, tag="rT")
            if emb_bias_ap is not None:
                nc.vector.tensor_scalar_add(rt[:], pp[:], emb_bias_ap[dc])
            else:
                nc.vector.tensor_copy(out=rt[:], in_=pp[:])
            resTc[tcix][dc] = rt
            if dc % 2 == 0:
                rT8c[tcix][dc // 2] = rT8_pool.tile([128, 2, CW], F8, tag="rT8", name="rt8e")
            if emb_bias_ap is not None:
                nc.vector.tensor_scalar(out=rT8c[tcix][dc // 2][:, dc % 2, :],
                                        in0=pp[:], scalar1=emb_bias_ap[dc],
                                        scalar2=a_sc(0),
                                        op0=mybir.AluOpType.add,
                                        op1=mybir.AluOpType.mult)
            else:
                nc.vector.tensor_scalar_mul(rT8c[tcix][dc // 2][:, dc % 2, :],
                                            pp[:], a_sc(0))

    mask_ap = None
    if "maskneg" in opt:
        mk = const.tile([128, TT], F32)
        nc.sync.dma_start(mk[:], opt["maskneg"].rearrange("(a p) -> p a", p=128))
        mask_ap = mk

    # ---- per-layer helpers --------------------------------------------
    QT_AHEAD = 6  # heads whose chunk-0 QT is prefetched at the end of the previous layer

    def load_ln_gb(li, nm):
        if nm + "_g" not in opt:
            return None
        gb = const.tile([128, 2, D], F32, tag=f"lngb{nm}{li}")
        nc.sync.dma_start(gb[:, 0, :], opt[nm + "_g"][li].partition_broadcast(128))
        nc.sync.dma_start(gb[:, 1, :], opt[nm + "_b"][li].partition_broadcast(128))
        return gb

    def layernorm(aps, gb):
        for x in aps:
            stats = sm_pool.tile([128, 3, 6], F32, tag="bnst")
            mv = sm_pool.tile([128, 2], F32, tag="bnmv")
            xg = x.rearrange("p (a c) -> p a c", a=3)
            for a in range(3):
                nc.vector.bn_stats(out=stats[:, a, :], in_=xg[:, a, :])
            nc.vector.bn_aggr(out=mv[:], in_=stats[:])
            rstd = sm_pool.tile([128, 1], F32, tag="rstd")
            nc.scalar.activation(out=rstd[:], in_=mv[:, 1:2],
                                 func=mybir.ActivationFunctionType.Sqrt,
                                 bias=eps_t[:], scale=1.0)
            nc.vector.reciprocal(rstd[:], rstd[:])
            nc.vector.tensor_scalar(out=x, in0=x, scalar1=mv[:, 0:1],
                                    scalar2=rstd[:],
                                    op0=mybir.AluOpType.subtract,
                                    op1=mybir.AluOpType.mult)
            if gb is not None:
                nc.vector.tensor_mul(x, x, gb[:, 0, :])
                nc.vector.tensor_add(x, x, gb[:, 1, :])

    def emit_kv_half(li, sc, h, rT8_l, resTc_l, kt_all, v_all):
        """KT s-half (fp8 DoubleRow) + V s-half (bf16) for one head.
        kt[(h,j)]: [128 k, 2, 512 s] fp8 pairs, v[(h,sm)]: [128 s, 768 k] bf16."""
        wk8 = load_wk8(li, h)
        wv_t = load_big(wv_d, li, h)
        dr_sc = qk_drain(li)
        for m2 in range(JD):
            pm = mm_tile()
            for half in range(2):
                m = 2 * m2 + half
                for j in range(JD):
                    nc.tensor.matmul(pm[:, half * CW:half * CW + CW],
                                     wk8[:, j, :, m * 128:(m + 1) * 128],
                                     rT8_l[sc][j][:],
                                     start=(j == 0), stop=(j == JD - 1),
                                     perf_mode=DR)
            if sc == 0:
                kt_all[(h, m2)] = kt_pool.tile([128, 2, S], F8, tag="kt",
                                               name=f"kt{h}_{m2}")
            pmv = pm[:].rearrange("p (i c) -> p i c", i=2)
            if m2 % 2 == 0:
                nc.scalar.mul(kt_all[(h, m2)][:, :, sc * CW:(sc + 1) * CW],
                              pmv, dr_sc)
            else:
                nc.vector.tensor_scalar_mul(
                    kt_all[(h, m2)][:, :, sc * CW:(sc + 1) * CW],
                    pmv, dr_sc)
        for tl in range(TPC):
            sm = sc * TPC + tl
            vt = v_pool.tile([128, D], BF16, tag="v")
            v_all[(h, sm)] = vt
            for (n0, nw) in NCH:
                pm = mm_tile()
                for dc in range(DC):
                    nc.tensor.matmul(pm[:, :nw],
                                     resTc_l[sc][dc][:, tl * 128:(tl + 1) * 128],
                                     wv_t[:, dc, n0:n0 + nw],
                                     start=(dc == 0), stop=(dc == DC - 1))
                nc.scalar.copy(out=vt[:, n0:n0 + nw], in_=pm[:, :nw])

    def emit_qt(li, tcix, h, rT8_l):
        """QT for one head/chunk via fp8 DoubleRow, packed 2 m's per PSUM bank.
        Returns 3 tiles [128, 512] fp8: tile j = m (2j, 2j+1) x 256 t."""
        wq8 = load_wq8(li, h)
        dr_sc = qk_drain(li)
        qt_sb = []
        for jo in range(JD):
            pm = mm_tile()
            for half in range(2):
                m = 2 * jo + half
                for j in range(JD):
                    nc.tensor.matmul(pm[:, half * CW:half * CW + CW],
                                     wq8[:, j, :, m * 128:(m + 1) * 128],
                                     rT8_l[tcix][j][:],
                                     start=(j == 0), stop=(j == JD - 1),
                                     perf_mode=DR)
            ot = qt_pool.tile([128, 512], F8, tag="qt")
            if jo % 2 == 0:
                nc.vector.tensor_scalar_mul(ot[:], pm[:], dr_sc)
            else:
                nc.scalar.mul(ot[:], pm[:], dr_sc)
            qt_sb.append(ot)
        return qt_sb

    def emit_scores(li, tcix, h, qt_sb, kt_all):
        """Transposed scores: peT[s, t] = exp(K Q^T) via fp8 DoubleRow, plus
        per-t 1/sum (applied later at the out-proj drain). Removes the whole
        P-transpose stage. pe_pair[smp] is [128 s, (2 sm-half)(256 t)] bf16 —
        the same layout the old pt_sb had, so ctx consumes it unchanged."""
        pe_pair = []
        for smp in range(2):
            pm = mm_tile()
            for half in range(2):
                sm = 2 * smp + half
                for j in range(JD):
                    qv = qt_sb[j][:].rearrange("p (i c) -> p i c", i=2)
                    nc.tensor.matmul(pm[:, half * CW:half * CW + CW],
                                     kt_all[(h, j)][:, :, sm * 128:(sm + 1) * 128],
                                     qv,
                                     start=(j == 0), stop=(j == JD - 1),
                                     perf_mode=DR)
                if mask_ap is not None:
                    nc.vector.tensor_scalar_add(pm[:, half * CW:half * CW + CW],
                                                pm[:, half * CW:half * CW + CW],
                                                mask_ap[:, sm:sm + 1])
            pe = pe_pool.tile([128, S], BF16, tag="pe")
            nc.scalar.activation(out=pe[:], in_=pm[:],
                                 func=mybir.ActivationFunctionType.Exp,
                                 scale=EXP_SCALE)
            pe_pair.append(pe)
        return (pe_pair,)

    def emit_sums(pe_pair):
        """per-t softmax denominators from peT via ones-matmuls (deferred so
        the PE queue isn't head-of-line blocked on the exp while dense work
        from the previous head is available)."""
        sp = ps_tp.tile([128, 2], F32, tag="tp", name="sumsps")
        for tb in range(TPC):
            k = 0
            for smp in range(2):
                for half in range(2):
                    nc.tensor.matmul(sp[:, tb:tb + 1],
                                     pe_pair[smp][:, half * CW + tb * 128:
                                                  half * CW + (tb + 1) * 128],
                                     ones_t[:], start=(k == 0), stop=(k == 3))
                    k += 1
        rec = sm_pool.tile([128, 2], F32, tag="rec")
        nc.vector.reciprocal(rec[:], sp[:])
        return rec

    def emit_ctx(li, tcix, h, pe_pair, v_all):
        """ctxT from unnormalized peT."""
        # ctxT packed: tile j holds km (2j | 2j+1) x 256 t (unnormalized)
        ct_sb = []
        for j in range(DC // 2):
            pm = mm_tile()
            for half in range(2):
                km = 2 * j + half
                for sm in range(TT):
                    nc.tensor.matmul(pm[:, half * CW:half * CW + CW],
                                     v_all[(h, sm)][:, km * 128:(km + 1) * 128],
                                     pe_pair[sm // 2][:, (sm % 2) * CW:(sm % 2) * CW + CW],
                                     start=(sm == 0), stop=(sm == TT - 1))
            ot = ct_pool.tile([128, 512], BF16, tag="ct")
            nc.vector.tensor_copy(out=ot[:], in_=pm[:])
            ct_sb.append(ot)
        return ct_sb

    def emit_out(li, tcix, h, ct_sb, rec, acc, accb):
        """out-proj partial with 1/sum folded into the accumulate as a
        per-t-partition scalar."""
        wo_t = load_big(wo_d, li, h)
        for tl in range(TPC):
            for (n0, nw) in NCH:
                pm = mm_tile()
                for kc in range(DC):
                    nc.tensor.matmul(pm[:, :nw],
                                     ct_sb[kc // 2][:, (kc % 2) * CW + tl * 128:
                                                    (kc % 2) * CW + (tl + 1) * 128],
                                     wo_t[:, kc, n0:n0 + nw],
                                     start=(kc == 0), stop=(kc == DC - 1))
                if h == 0:
                    nc.vector.tensor_scalar_mul(acc[tl][:, n0:n0 + nw],
                                                pm[:, :nw], rec[:, tl:tl + 1])
                elif h < HPC - 1:
                    nc.vector.scalar_tensor_tensor(
                        out=acc[tl][:, n0:n0 + nw], in0=pm[:, :nw],
                        scalar=rec[:, tl:tl + 1], in1=acc[tl][:, n0:n0 + nw],
                        op0=mybir.AluOpType.mult, op1=mybir.AluOpType.add)
                else:
                    nc.vector.scalar_tensor_tensor(
                        out=accb[:, tl, n0:n0 + nw], in0=pm[:, :nw],
                        scalar=rec[:, tl:tl + 1], in1=acc[tl][:, n0:n0 + nw],
                        op0=mybir.AluOpType.mult, op1=mybir.AluOpType.add)

    def emit_collective(li, accb):
        arin = dram.tile([CW, D], BF16, tag="arin")
        last = li == n_layers - 1
        nc.sync.dma_start(arin[:].rearrange("(a p) d -> p a d", p=128),
                          accb[:, 0:TPC, :])
        if last:
            arout = dram.tile([128, D], BF16, tag="arout2")
            nc.gpsimd.collective_compute(
                "ReduceScatter", mybir.AluOpType.add,
                replica_groups=[[0, 1], [2, 3], [4, 5], [6, 7]],
                ins=[arin.opt()], outs=[arout.opt()])
        else:
            # AllGather both partials (no AllReduce cost multiplier on the
            # collective cores); the pairwise add happens locally on DVE.
            arout = dram.tile([2 * CW, D], BF16, tag="arout")
            nc.gpsimd.collective_compute(
                "AllGather", mybir.AluOpType.bypass,
                replica_groups=[[0, 1], [2, 3], [4, 5], [6, 7]],
                ins=[arin.opt()], outs=[arout.opt()])
        return arout

    def emit_tail_chunk(li, tcix, arout, gb1, gb2, ff_t, resTc_next, rT8_next):
        """AR result -> LN1 -> FFN -> LN2 -> resTc_next[tcix] (or output DMA).
        For the last layer the collective was a ReduceScatter: each core owns
        128 of the 256 chunk rows; the host reassembles."""
        ntl = 1 if li == n_layers - 1 else TPC
        xc = accb_pool.tile([128, TPC, D], BF16, tag="accb", name="xcur")
        if li == n_layers - 1:
            nc.gpsimd.dma_start(xc[:, 0:ntl, :],
                                arout[:].rearrange("(a p) d -> p a d", p=128))
        else:
            xg = wbig_pool.tile([128, 2, TPC, D], BF16, tag="wbig", name="xg")
            nc.gpsimd.dma_start(xg[:],
                                arout[:].rearrange("(g a p) d -> p g a d", p=128, g=2))
            nc.vector.tensor_add(xc[:, 0:ntl, :], xg[:, 0, :, :], xg[:, 1, :, :])
        xcur = [xc[:, tl, :] for tl in range(ntl)]
        layernorm(xcur, gb1)

        lnT = []
        for dc in range(DC):
            pp = ps_tp.tile([128, CW], BF16, tag="tp", name="tpps")
            for tl in range(ntl):
                nc.tensor.transpose(pp[:, tl * 128:(tl + 1) * 128],
                                    xc[:, tl, dc * 128:(dc + 1) * 128],
                                    identb[:])
            t = pt_pool.tile([128, 512], BF16, tag="pts", name="lnT")
            nc.scalar.copy(out=t[:, :ntl * 128], in_=pp[:, :ntl * 128])
            lnT.append(t)

        xmid = [xtd_pool.tile([128, D], F32, tag="xtd", name=f"xmid{tl}") for tl in range(ntl)]
        xmid_ap = [t[:] for t in xmid]
        for tl in range(ntl):
            for (n0, nw) in NCH:
                pm = mm_tile()
                for dc in range(DC):
                    nc.tensor.matmul(pm[:, :nw], lnT[dc][:, tl * 128:(tl + 1) * 128],
                                     ff_t[:, dc, n0:n0 + nw],
                                     start=(dc == 0), stop=(dc == DC - 1))
                nc.vector.tensor_copy(out=xmid[tl][:, n0:n0 + nw], in_=pm[:, :nw])

        layernorm(xmid_ap, gb2)

        if li < n_layers - 1:
            for dc in range(DC):
                pp = ps_tp.tile([128, CW], F32, tag="tp", name="tpps")
                for tl in range(TPC):
                    nc.tensor.transpose(pp[:, tl * 128:(tl + 1) * 128],
                                        xmid[tl][:, dc * 128:(dc + 1) * 128],
                                        ident[:])
                rt = rT_pool.tile([128, CW], BF16, tag="rT")
                nc.scalar.copy(out=rt[:], in_=pp[:])
                resTc_next[tcix][dc] = rt
                if dc % 2 == 0:
                    rT8_next[tcix][dc // 2] = rT8_pool.tile([128, 2, CW], F8, tag="rT8", name="rt8n")
                nc.vector.tensor_scalar_mul(rT8_next[tcix][dc // 2][:, dc % 2, :],
                                            pp[:], a_sc(li + 1))
        else:
            nc.sync.dma_start(out_d[tcix * 128:(tcix + 1) * 128, :], xmid[0][:])

    # ---- layers --------------------------------------------------------
    # prologue: layer-0 chunk-0 KV + QT prefetch (resTc from embeddings)
    cur_kt, cur_v = {}, {}
    qt_pre = {}
    for h in range(HPC):
        emit_kv_half(0, 0, h, rT8c, resTc, cur_kt, cur_v)
        if h < QT_AHEAD:
            qt_pre[h] = emit_qt(0, 0, h, rT8c)

    for li in range(n_layers):
        gb1 = load_ln_gb(li, "ln1")
        gb2 = load_ln_gb(li, "ln2")

        # A: KV s-half 1 (skewed) + chunk-0 score chains
        acc0 = [xtd_pool.tile([128, D], F32, tag="xtd", name=f"acc{tl}") for tl in range(TPC)]
        accb0 = accb_pool.tile([128, TPC, D], BF16, tag="accb", name="accb0")
        emit_kv_half(li, 1, 0, rT8c, resTc, cur_kt, cur_v)
        pend = None
        for h in range(HPC):
            qt_sb = qt_pre.pop(h) if h in qt_pre else emit_qt(li, 0, h, rT8c)
            if h + 1 < HPC:
                emit_kv_half(li, 1, h + 1, rT8c, resTc, cur_kt, cur_v)
            (pe_pair,) = emit_scores(li, 0, h, qt_sb, cur_kt)
            if pend is not None:
                ct_prev = emit_ctx(li, 0, pend[0], pend[1], cur_v)
            rec = emit_sums(pe_pair)
            if pend is not None:
                emit_out(li, 0, pend[0], ct_prev, pend[2], acc0, accb0)
            pend = (h, pe_pair, rec)
        ct_prev = emit_ctx(li, 0, pend[0], pend[1], cur_v)
        emit_out(li, 0, pend[0], ct_prev, pend[2], acc0, accb0)
        arout0 = emit_collective(li, accb0)

        # C: chunk-1 score chains (AR(c0) overlaps this)
        acc1 = [xtd_pool.tile([128, D], F32, tag="xtd", name=f"acc{tl}") for tl in range(TPC)]
        accb1 = accb_pool.tile([128, TPC, D], BF16, tag="accb", name="accb1")
        pend = None
        for h in range(HPC):
            qt_sb = emit_qt(li, 1, h, rT8c)
            if pend is not None:
                ct_prev = emit_ctx(li, 1, pend[0], pend[1], cur_v)
            (pe_pair,) = emit_scores(li, 1, h, qt_sb, cur_kt)
            if pend is not None:
                emit_out(li, 1, pend[0], ct_prev, pend[2], acc1, accb1)
            rec = emit_sums(pe_pair)
            pend = (h, pe_pair, rec)
        ct_prev = emit_ctx(li, 1, pend[0], pend[1], cur_v)
        emit_out(li, 1, pend[0], ct_prev, pend[2], acc1, accb1)

        ff_t = load_big(ff_d, li)

        # E: tail chunk 0 — emitted before the chunk-1 collective so its
        # gathered-load/add/LN serial chain runs during C on the free engines
        resTc_next = [[None] * DC for _ in range(NTC)]
        rT8_next = [[None] * JD for _ in range(NTC)]
        emit_tail_chunk(li, 0, arout0, gb1, gb2, ff_t, resTc_next, rT8_next)
        arout1 = emit_collective(li, accb1)

        # F: next layer's chunk-0 KV + QT prefetch (fills AR(c1) window)
        next_kt, next_v = {}, {}
        qt_pre = {}
        if li < n_layers - 1:
            for h in range(HPC):
                emit_kv_half(li + 1, 0, h, rT8_next, resTc_next, next_kt, next_v)
                if h < QT_AHEAD:
                    qt_pre[h] = emit_qt(li + 1, 0, h, rT8_next)

        # G: tail chunk 1
        emit_tail_chunk(li, 1, arout1, gb1, gb2, ff_t, resTc_next, rT8_next)

        resTc = resTc_next
        rT8c = rT8_next
        cur_kt, cur_v = next_kt, next_v


# ------------------------------------------------------------------------
# host side
# ------------------------------------------------------------------------
_CACHED = {}
_LAST_RES = None


def _get_nc(n_layers, flag_key, flags):
    key = (n_layers, flag_key)
    if key not in _CACHED:
        _CACHED[key] = build_nc(n_layers, flags)
    return _CACHED[key]


def _fp8(x, scale):
    return np.clip(x * scale, -240.0, 240.0).astype(E4)


def kernel(X, tok_w, tok_b, pos_w, pos_b, seg_w, seg_b,
           Wq, bq, Wk, bk, Wv, bv, Wo, bo,
           ln1_g, ln1_b, ffp_w, ffp_b, ln2_g, ln2_b, n_layers=L):
    global _LAST_RES
    f32 = np.float32
    X = np.asarray(X, dtype=np.int32)
    tok_w = np.asarray(tok_w, f32); pos_w = np.asarray(pos_w, f32); seg_w = np.asarray(seg_w, f32)
    Wq = np.asarray(Wq, f32); Wk = np.asarray(Wk, f32); Wv = np.asarray(Wv, f32)
    Wo = np.asarray(Wo, f32); ffp_w = np.asarray(ffp_w, f32)
    bq = np.asarray(bq, f32); bk = np.asarray(bk, f32); bv = np.asarray(bv, f32)
    bo = np.asarray(bo, f32); ffp_b = np.asarray(ffp_b, f32)
    ln1_g = np.asarray(ln1_g, f32); ln1_b = np.asarray(ln1_b, f32)
    ln2_g = np.asarray(ln2_g, f32); ln2_b = np.asarray(ln2_b, f32)
    tok_b = np.asarray(tok_b, f32); pos_b = np.asarray(pos_b, f32); seg_b = np.asarray(seg_b, f32)

    emb_bias = tok_b + pos_b + seg_b
    flags = {
        "emb_bias": bool(np.any(emb_bias)),
        "ln1": bool(np.any(ln1_g != 1) or np.any(ln1_b)),
        "ln2": bool(np.any(ln2_g != 1) or np.any(ln2_b)),
        "mask": bool(np.any(X[:, 0, :] == 0)),
    }
    assert not (np.any(bo) or np.any(ffp_b) or np.any(bq) or np.any(bk) or np.any(bv)), \
        "nonzero attention/ffn biases not implemented in this specialization"
    flag_key = tuple(sorted(flags.items()))
    nc = _get_nc(n_layers, flag_key, flags)

    tok_wb = tok_w.astype(BF)
    pos_wb = pos_w.astype(BF)
    seg_wb = seg_w.astype(BF)

    in_maps = []
    per_g = {}
    nl = n_layers
    for g in range(2):
        hsl = slice(g * HPC, (g + 1) * HPC)
        # [L, D, HK] per-group flattened weights
        wq_f = np.ascontiguousarray(Wq[:nl, :, hsl, :]).reshape(nl, D, HK)
        wk_f = np.ascontiguousarray(Wk[:nl, :, hsl, :]).reshape(nl, D, HK)
        wv_f = np.ascontiguousarray(Wv[:nl, :, hsl, :]).reshape(nl, D, HK)
        wo_f = np.ascontiguousarray(Wo[:nl, hsl, :, :]).reshape(nl, HK, D)
        # fp8 pair layout [L, h, p, j, i, 768]
        wq8 = np.ascontiguousarray(
            _fp8(wq_f, W_SC).reshape(nl, JD, 2, 128, HPC, KH).transpose(0, 4, 3, 1, 2, 5))
        wk8 = np.ascontiguousarray(
            _fp8(wk_f, W_SC).reshape(nl, JD, 2, 128, HPC, KH).transpose(0, 4, 3, 1, 2, 5))
        # bf16 per-head layouts
        wvh = np.ascontiguousarray(
            wv_f.astype(BF).reshape(nl, DC, 128, HPC, KH).transpose(0, 3, 2, 1, 4))
        woh = np.ascontiguousarray(
            wo_f.astype(BF).reshape(nl, HPC, DC, 128, D).transpose(0, 1, 3, 2, 4))
        per_g[g] = {"wq8": wq8, "wk8": wk8, "wv": wvh, "wo": woh}
    ffh = np.ascontiguousarray(
        ffp_w[:nl].astype(BF).reshape(nl, DC, 128, D).transpose(0, 2, 1, 3))

    for c in range(NCORES):
        b, g = c // 2, c % 2
        m = {
            "xids": np.ascontiguousarray(X[b]),
            "tok_w": tok_wb, "pos_w": pos_wb, "seg_w": seg_wb,
            "ff": ffh,
            **per_g[g],
        }
        if flags["emb_bias"]:
            m["emb_bias"] = emb_bias
        if flags["ln1"]:
            m["ln1_g"] = np.ascontiguousarray(ln1_g[:nl])
            m["ln1_b"] = np.ascontiguousarray(ln1_b[:nl])
        if flags["ln2"]:
            m["ln2_g"] = np.ascontiguousarray(ln2_g[:nl])
            m["ln2_b"] = np.ascontiguousarray(ln2_b[:nl])
        if flags["mask"]:
            m["maskneg"] = np.where(X[b, 0, :] == 0, -1e9 * QK_SC * QK_SC, 0.0).astype(f32)
        in_maps.append(m)

    res = bass_utils.run_bass_kernel_spmd(nc, in_maps, core_ids=list(range(NCORES)))
    _LAST_RES = res
    out = np.empty((B, S, D), np.float32)
    for b in range(B):
        o0 = res.results[2 * b]["out"]      # rank-0 shards: rows 0:128 / 256:384
        o1 = res.results[2 * b + 1]["out"]  # rank-1 shards: rows 128:256 / 384:512
        out[b, 0:128] = o0[0:128]
        out[b, 128:256] = o1[0:128]
        out[b, 256:384] = o0[128:256]
        out[b, 384:512] = o1[128:256]
    return out


# revision 6
# speedup vs baseline: 1.0610x; 1.0019x over previous
"""Trainium2 Bass kernel for nn_JslBERT — v2: fp8 DoubleRow Q/K/scores.

Sharding: 8 cores = 4 batch x 2 head-groups (6 heads each). Per layer,
attention-output partials are pairwise AllReduced in bf16; LN+FFN run
redundantly on both cores of a pair.

v2 changes vs baseline:
 - Q-proj, K-proj and QK^T scores run in fp8(e4m3) with perf_mode=DoubleRow
   (contraction pairs of 128-chunks per instruction). Static power-of-2
   scaling: weights x1024, residual x16 (x512 in layer 0), Q^T/K^T stored
   x32; descale folded into the PSUM drains and the exp() scale.
   V/ctx/out-proj/FFN stay bf16 (precision-critical path; fp8 there blows
   the 2e-2 gate).
 - Weight DMAs consolidated: one DMA per (tensor, head[, chunk]) instead of
   one per 128-row d-chunk (HWDGE instruction-count was near saturation).
"""
import os
import numpy as np
import ml_dtypes

import concourse.bass as bass
import concourse.bacc as bacc
import concourse.tile as tile
import concourse.bass_utils as bass_utils
from concourse import mybir
from concourse.masks import make_identity

# Model dims (hardcoded per problem spec)
B, S, L, D, H, V, PMAX = 4, 512, 4, 768, 12, 32000, 512
EPS = 1e-3
NCORES = 8
HPC = H // 2          # heads per core
KH = D                # head dim (768)
HK = HPC * KH         # 4608 flattened head dims per core
SCALE = 1.0 / float(np.sqrt(D))

F32 = mybir.dt.float32
BF16 = mybir.dt.bfloat16
F8 = mybir.dt.float8e4
I32 = mybir.dt.int32
DR = mybir.MatmulPerfMode.DoubleRow

TT = S // 128         # 4 t-tiles total
DC = D // 128         # 6 d-chunks
JD = DC // 2          # 3 d-chunk pairs (DoubleRow)
NCH = [(0, 512), (512, 256)]  # free-dim chunks for width-768 outputs
NTC = 2               # t-chunks per sequence
TPC = TT // NTC       # 128-tiles per chunk (2)
CW = S // NTC         # chunk width (256)

# fp8 static scales
W_SC = 1024.0                     # wq/wk weights
QK_SC = 32.0                      # stored Q^T / K^T
EXP_SCALE = SCALE / (QK_SC * QK_SC)


def a_sc(li):                     # residual-stream fp8 scale
    return 512.0 if li == 0 else 16.0


def qk_drain(li):                 # PSUM -> qt/kt fp8 multiplier
    return QK_SC / (a_sc(li) * W_SC)


BF = np.dtype(ml_dtypes.bfloat16)
E4 = np.dtype(ml_dtypes.float8_e4m3)


def build_nc(n_layers=L, flags=None):
    """Build the Bass graph. flags: dict of which optional inputs exist."""
    flags = flags or {}
    nc = bacc.Bacc("TRN2", target_bir_lowering=False, debug=False,
                   num_devices=NCORES)

    xids_d = nc.dram_tensor("xids", [3, S], I32, kind="ExternalInput").ap()
    tokw_d = nc.dram_tensor("tok_w", [V, D], BF16, kind="ExternalInput").ap()
    posw_d = nc.dram_tensor("pos_w", [PMAX, D], BF16, kind="ExternalInput").ap()
    segw_d = nc.dram_tensor("seg_w", [2, D], BF16, kind="ExternalInput").ap()
    # fp8 pair layouts: [L, h, p, j, i, 768] with slot i = d-chunk 2j+i
    wq_d = nc.dram_tensor("wq8", [n_layers, HPC, 128, JD, 2, KH], F8, kind="ExternalInput").ap()
    wk_d = nc.dram_tensor("wk8", [n_layers, HPC, 128, JD, 2, KH], F8, kind="ExternalInput").ap()
    # bf16 per-head layouts: [L, h, p, dc|kc, 768]
    wv_d = nc.dram_tensor("wv", [n_layers, HPC, 128, DC, KH], BF16, kind="ExternalInput").ap()
    wo_d = nc.dram_tensor("wo", [n_layers, HPC, 128, DC, D], BF16, kind="ExternalInput").ap()
    ff_d = nc.dram_tensor("ff", [n_layers, 128, DC, D], BF16, kind="ExternalInput").ap()
    out_d = nc.dram_tensor("out", [S, D], F32, kind="ExternalOutput").ap()

    opt = {}
    if flags.get("emb_bias"):
        opt["emb_bias"] = nc.dram_tensor("emb_bias", [D], F32, kind="ExternalInput").ap()
    for nm in ("ln1", "ln2"):
        if flags.get(nm):
            opt[nm + "_g"] = nc.dram_tensor(nm + "_g", [n_layers, D], F32, kind="ExternalInput").ap()
            opt[nm + "_b"] = nc.dram_tensor(nm + "_b", [n_layers, D], F32, kind="ExternalInput").ap()
    if flags.get("mask"):
        opt["maskneg"] = nc.dram_tensor("maskneg", [S], F32, kind="ExternalInput").ap()

    with tile.TileContext(nc) as tc:
        import contextlib
        with contextlib.ExitStack() as ctx:
            _build_body(ctx, tc, n_layers, flags, xids_d, tokw_d, posw_d, segw_d,
                        wq_d, wk_d, wv_d, wo_d, ff_d, out_d, opt)
    nc.compile()
    return nc


def _build_body(ctx, tc, n_layers, flags, xids_d, tokw_d, posw_d, segw_d,
                wq_d, wk_d, wv_d, wo_d, ff_d, out_d, opt):
    nc = tc.nc

    const = ctx.enter_context(tc.tile_pool(name="const", bufs=1))
    wq8_pool = ctx.enter_context(tc.tile_pool(name="wq8", bufs=3))
    wk8_pool = ctx.enter_context(tc.tile_pool(name="wk8", bufs=3))
    wbig_pool = ctx.enter_context(tc.tile_pool(name="wbig", bufs=3))   # wv/wo
    ff_pool = ctx.enter_context(tc.tile_pool(name="ffp", bufs=1))
    rT8_pool = ctx.enter_context(tc.tile_pool(name="rT8", bufs=8))
    rT_pool = ctx.enter_context(tc.tile_pool(name="rT", bufs=13))
    kt_pool = ctx.enter_context(tc.tile_pool(name="ktp", bufs=18))
    v_pool = ctx.enter_context(tc.tile_pool(name="vp", bufs=24))
    qt_pool = ctx.enter_context(tc.tile_pool(name="qtp", bufs=19))
    pe_pool = ctx.enter_context(tc.tile_pool(name="pep", bufs=5))
    pt_pool = ctx.enter_context(tc.tile_pool(name="ptp", bufs=8))
    ct_pool = ctx.enter_context(tc.tile_pool(name="ctp", bufs=5))
    xtd_pool = ctx.enter_context(tc.tile_pool(name="xtd", bufs=6))
    accb_pool = ctx.enter_context(tc.tile_pool(name="accb", bufs=4))
    sm_pool = ctx.enter_context(tc.tile_pool(name="sm", bufs=8))
    ps_mm = ctx.enter_context(tc.tile_pool(name="psmm", bufs=6, space="PSUM"))
    ps_tp = ctx.enter_context(tc.tile_pool(name="pstp", bufs=2, space="PSUM"))
    dram = ctx.enter_context(tc.tile_pool(name="dram", bufs=2, space="DRAM"))

    ident = const.tile([128, 128], F32)
    make_identity(nc, ident[:])
    identb = const.tile([128, 128], BF16)
    make_identity(nc, identb[:])
    eps_t = const.tile([128, 1], F32)
    nc.vector.memset(eps_t[:], EPS)
    ones_t = const.tile([128, 1], BF16)
    nc.vector.memset(ones_t[:], 1.0)

    def mm_tile():
        return ps_mm.tile([128, 512], F32, tag="mm", name="mmps")

    # ---- weight loaders ------------------------------------------------
    def load_wq8(li, h):
        t = wq8_pool.tile([128, JD, 2, KH], F8, tag="wq8")
        nc.sync.dma_start(t[:], wq_d[li, h])
        return t

    def load_wk8(li, h):
        t = wk8_pool.tile([128, JD, 2, KH], F8, tag="wk8")
        nc.sync.dma_start(t[:], wk_d[li, h])
        return t

    def load_big(wd, li, h=None):
        pool = wbig_pool if h is not None else ff_pool
        t = pool.tile([128, DC, D], BF16, tag="wbig" if h is not None else "ff")
        nc.sync.dma_start(t[:], wd[li] if h is None else wd[li, h])
        return t

    # ---- embeddings ----------------------------------------------------
    idx = const.tile([128, 3, TT], I32)
    nc.sync.dma_start(idx[:], xids_d.rearrange("k (j p) -> p k j", p=128))

    emb_bias_ap = None
    if "emb_bias" in opt:
        eb = const.tile([128, DC], F32)
        nc.sync.dma_start(eb[:], opt["emb_bias"].rearrange("(c p) -> p c", p=128))
        emb_bias_ap = [eb[:, c:c + 1] for c in range(DC)]

    # pos ids are structurally arange(S) (built that way in the model), so the
    # pos "lookup" is a direct row DMA; tok/seg stay data-dependent gathers.
    tok_t = wbig_pool.tile([128, DC, D], BF16, tag="wbig", name="tokt")
    pos_t = wbig_pool.tile([128, DC, D], BF16, tag="wbig", name="post")
    seg_t = wbig_pool.tile([128, DC, D], BF16, tag="wbig", name="segt")
    nc.sync.dma_start(pos_t[:, 0:TT, :],
                      posw_d.rearrange("(a p) d -> p a d", p=128))
    for tm in range(TT):
        nc.gpsimd.indirect_dma_start(
            out=tok_t[:, tm, :], out_offset=None, in_=tokw_d[:],
            in_offset=bass.IndirectOffsetOnAxis(ap=idx[:, 0, tm:tm + 1], axis=0))
        nc.gpsimd.indirect_dma_start(
            out=seg_t[:, tm, :], out_offset=None, in_=segw_d[:],
            in_offset=bass.IndirectOffsetOnAxis(ap=idx[:, 2, tm:tm + 1], axis=0))
    for tm in range(TT):
        nc.vector.tensor_add(tok_t[:, tm, :], tok_t[:, tm, :], pos_t[:, tm, :])
        nc.vector.tensor_add(tok_t[:, tm, :], tok_t[:, tm, :], seg_t[:, tm, :])

    # resTc[tcix][dc]: [128 d, 256 t] bf16 (V-proj stationary)
    # rT8c[tcix][j]:  [128 d, 2, 256 t] fp8 pairs (Q/K moving operand)
    resTc = [[None] * DC for _ in range(NTC)]
    rT8c = [[None] * JD for _ in range(NTC)]
    for tcix in range(NTC):
        for dc in range(DC):
            pp = ps_tp.tile([128, CW], BF16, tag="tp", name="tpps")
            for tl in range(TPC):
                nc.tensor.transpose(pp[:, tl * 128:(tl + 1) * 128],
                                    tok_t[:, tcix * TPC + tl, dc * 128:(dc + 1) * 128],
                                    identb[:])
            rt = rT_pool.tile([128, CW], BF16, tag="rT")
            if emb_bias_ap is not None:
                nc.vector.tensor_scalar_add(rt[:], pp[:], emb_bias_ap[dc])
            else:
                nc.vector.tensor_copy(out=rt[:], in_=pp[:])
            resTc[tcix][dc] = rt
            if dc % 2 == 0:
                rT8c[tcix][dc // 2] = rT8_pool.tile([128, 2, CW], F8, tag="rT8", name="rt8e")
            if emb_bias_ap is not None:
                nc.vector.tensor_scalar(out=rT8c[tcix][dc // 2][:, dc % 2, :],
                                        in0=pp[:], scalar1=emb_bias_ap[dc],
                                        scalar2=a_sc(0),
                                        op0=mybir.AluOpType.add,
                                        op1=mybir.AluOpType.mult)
            else:
                nc.vector.tensor_scalar_mul(rT8c[tcix][dc // 2][:, dc % 2, :],
                                            pp[:], a_sc(0))

    mask_ap = None
    if "maskneg" in opt:
        mk = const.tile([128, TT], F32)
        nc.sync.dma_start(mk[:], opt["maskneg"].rearrange("(a p) -> p a", p=128))
        mask_ap = mk

    # ---- per-layer helpers --------------------------------------------
    QT_AHEAD = 6  # heads whose chunk-0 QT is prefetched at the end of the previous layer

    def load_ln_gb(li, nm):
        if nm + "_g" not in opt:
            return None
        gb = const.tile([128, 2, D], F32, tag=f"lngb{nm}{li}")
        nc.sync.dma_start(gb[:, 0, :], opt[nm + "_g"][li].partition_broadcast(128))
        nc.sync.dma_start(gb[:, 1, :], opt[nm + "_b"][li].partition_broadcast(128))
        return gb

    def layernorm(aps, gb):
        for x in aps:
            stats = sm_pool.tile([128, 3, 6], F32, tag="bnst")
            mv = sm_pool.tile([128, 2], F32, tag="bnmv")
            xg = x.rearrange("p (a c) -> p a c", a=3)
            for a in range(3):
                nc.vector.bn_stats(out=stats[:, a, :], in_=xg[:, a, :])
            nc.vector.bn_aggr(out=mv[:], in_=stats[:])
            rstd = sm_pool.tile([128, 1], F32, tag="rstd")
            nc.scalar.activation(out=rstd[:], in_=mv[:, 1:2],
                                 func=mybir.ActivationFunctionType.Sqrt,
                                 bias=eps_t[:], scale=1.0)
            nc.vector.reciprocal(rstd[:], rstd[:])
            nc.vector.tensor_scalar(out=x, in0=x, scalar1=mv[:, 0:1],
                                    scalar2=rstd[:],
                                    op0=mybir.AluOpType.subtract,
                                    op1=mybir.AluOpType.mult)
            if gb is not None:
                nc.vector.tensor_mul(x, x, gb[:, 0, :])
                nc.vector.tensor_add(x, x, gb[:, 1, :])

    def emit_kv_half(li, sc, h, rT8_l, resTc_l, kt_all, v_all):
        """KT s-half (fp8 DoubleRow) + V s-half (bf16) for one head.
        kt[(h,j)]: [128 k, 2, 512 s] fp8 pairs, v[(h,sm)]: [128 s, 768 k] bf16."""
        wk8 = load_wk8(li, h)
        wv_t = load_big(wv_d, li, h)
        dr_sc = qk_drain(li)
        for m2 in range(JD):
            pm = mm_tile()
            for half in range(2):
                m = 2 * m2 + half
                for j in range(JD):
                    nc.tensor.matmul(pm[:, half * CW:half * CW + CW],
                                     wk8[:, j, :, m * 128:(m + 1) * 128],
                                     rT8_l[sc][j][:],
                                     start=(j == 0), stop=(j == JD - 1),
                                     perf_mode=DR)
            if sc == 0:
                kt_all[(h, m2)] = kt_pool.tile([128, 2, S], F8, tag="kt",
                                               name=f"kt{h}_{m2}")
            pmv = pm[:].rearrange("p (i c) -> p i c", i=2)
            if m2 % 2 == 0:
                nc.scalar.mul(kt_all[(h, m2)][:, :, sc * CW:(sc + 1) * CW],
                              pmv, dr_sc)
            else:
                nc.vector.tensor_scalar_mul(
                    kt_all[(h, m2)][:, :, sc * CW:(sc + 1) * CW],
                    pmv, dr_sc)
        for tl in range(TPC):
            sm = sc * TPC + tl
            vt = v_pool.tile([128, D], BF16, tag="v")
            v_all[(h, sm)] = vt
            for (n0, nw) in NCH:
                pm = mm_tile()
                for dc in range(DC):
                    nc.tensor.matmul(pm[:, :nw],
                                     resTc_l[sc][dc][:, tl * 128:(tl + 1) * 128],
                                     wv_t[:, dc, n0:n0 + nw],
                                     start=(dc == 0), stop=(dc == DC - 1))
                nc.scalar.copy(out=vt[:, n0:n0 + nw], in_=pm[:, :nw])

    def emit_qt(li, tcix, h, rT8_l):
        """QT for one head/chunk via fp8 DoubleRow, packed 2 m's per PSUM bank.
        Returns 3 tiles [128, 512] fp8: tile j = m (2j, 2j+1) x 256 t."""
        wq8 = load_wq8(li, h)
        dr_sc = qk_drain(li)
        qt_sb = []
        for jo in range(JD):
            pm = mm_tile()
            for half in range(2):
                m = 2 * jo + half
                for j in range(JD):
                    nc.tensor.matmul(pm[:, half * CW:half * CW + CW],
                                     wq8[:, j, :, m * 128:(m + 1) * 128],
                                     rT8_l[tcix][j][:],
                                     start=(j == 0), stop=(j == JD - 1),
                                     perf_mode=DR)
            ot = qt_pool.tile([128, 512], F8, tag="qt")
            if jo % 2 == 0:
                nc.vector.tensor_scalar_mul(ot[:], pm[:], dr_sc)
            else:
                nc.scalar.mul(ot[:], pm[:], dr_sc)
            qt_sb.append(ot)
        return qt_sb

    def emit_scores(li, tcix, h, qt_sb, kt_all):
        """Transposed scores: peT[s, t] = exp(K Q^T) via fp8 DoubleRow, plus
        per-t 1/sum (applied later at the out-proj drain). Removes the whole
        P-transpose stage. pe_pair[smp] is [128 s, (2 sm-half)(256 t)] bf16 —
        the same layout the old pt_sb had, so ctx consumes it unchanged."""
        pe_pair = []
        for smp in range(2):
            pm = mm_tile()
            for half in range(2):
                sm = 2 * smp + half
                for j in range(JD):
                    qv = qt_sb[j][:].rearrange("p (i c) -> p i c", i=2)
                    nc.tensor.matmul(pm[:, half * CW:half * CW + CW],
                                     kt_all[(h, j)][:, :, sm * 128:(sm + 1) * 128],
                                     qv,
                                     start=(j == 0), stop=(j == JD - 1),
                                     perf_mode=DR)
                if mask_ap is not None:
                    nc.vector.tensor_scalar_add(pm[:, half * CW:half * CW + CW],
                                                pm[:, half * CW:half * CW + CW],
                                                mask_ap[:, sm:sm + 1])
            pe = pe_pool.tile([128, S], BF16, tag="pe")
            nc.scalar.activation(out=pe[:], in_=pm[:],
                                 func=mybir.ActivationFunctionType.Exp,
                                 scale=EXP_SCALE)
            pe_pair.append(pe)
        return (pe_pair,)

    def emit_sums(pe_pair):
        """per-t softmax denominators from peT via ones-matmuls (deferred so
        the PE queue isn't head-of-line blocked on the exp while dense work
        from the previous head is available)."""
        sp = ps_tp.tile([128, 2], F32, tag="tp", name="sumsps")
        for tb in range(TPC):
            k = 0
            for smp in range(2):
                for half in range(2):
                    nc.tensor.matmul(sp[:, tb:tb + 1],
                                     pe_pair[smp][:, half * CW + tb * 128:
                                                  half * CW + (tb + 1) * 128],
                                     ones_t[:], start=(k == 0), stop=(k == 3))
                    k += 1
        rec = sm_pool.tile([128, 2], F32, tag="rec")
        nc.vector.reciprocal(rec[:], sp[:])
        return rec

    def emit_ctx(li, tcix, h, pe_pair, v_all):
        """ctxT from unnormalized peT."""
        # ctxT packed: tile j holds km (2j | 2j+1) x 256 t (unnormalized)
        ct_sb = []
        for j in range(DC // 2):
            pm = mm_tile()
            for half in range(2):
                km = 2 * j + half
                for sm in range(TT):
                    nc.tensor.matmul(pm[:, half * CW:half * CW + CW],
                                     v_all[(h, sm)][:, km * 128:(km + 1) * 128],
                                     pe_pair[sm // 2][:, (sm % 2) * CW:(sm % 2) * CW + CW],
                                     start=(sm == 0), stop=(sm == TT - 1))
            ot = ct_pool.tile([128, 512], BF16, tag="ct")
            nc.vector.tensor_copy(out=ot[:], in_=pm[:])
            ct_sb.append(ot)
        return ct_sb

    def emit_out(li, tcix, h, ct_sb, rec, acc, accb):
        """out-proj partial with 1/sum folded into the accumulate as a
        per-t-partition scalar."""
        wo_t = load_big(wo_d, li, h)
        for tl in range(TPC):
            for (n0, nw) in NCH:
                pm = mm_tile()
                for kc in range(DC):
                    nc.tensor.matmul(pm[:, :nw],
                                     ct_sb[kc // 2][:, (kc % 2) * CW + tl * 128:
                                                    (kc % 2) * CW + (tl + 1) * 128],
                                     wo_t[:, kc, n0:n0 + nw],
                                     start=(kc == 0), stop=(kc == DC - 1))
                if h == 0:
                    nc.vector.tensor_scalar_mul(acc[tl][:, n0:n0 + nw],
                                                pm[:, :nw], rec[:, tl:tl + 1])
                elif h < HPC - 1:
                    nc.vector.scalar_tensor_tensor(
                        out=acc[tl][:, n0:n0 + nw], in0=pm[:, :nw],
                        scalar=rec[:, tl:tl + 1], in1=acc[tl][:, n0:n0 + nw],
                        op0=mybir.AluOpType.mult, op1=mybir.AluOpType.add)
                else:
                    nc.vector.scalar_tensor_tensor(
                        out=accb[:, tl, n0:n0 + nw], in0=pm[:, :nw],
                        scalar=rec[:, tl:tl + 1], in1=acc[tl][:, n0:n0 + nw],
                        op0=mybir.AluOpType.mult, op1=mybir.AluOpType.add)

    def emit_collective(li, accb):
        arin = dram.tile([CW, D], BF16, tag="arin")
        last = li == n_layers - 1
        nc.sync.dma_start(arin[:].rearrange("(a p) d -> p a d", p=128),
                          accb[:, 0:TPC, :])
        if last:
            arout = dram.tile([128, D], BF16, tag="arout2")
            nc.gpsimd.collective_compute(
                "ReduceScatter", mybir.AluOpType.add,
                replica_groups=[[0, 1], [2, 3], [4, 5], [6, 7]],
                ins=[arin.opt()], outs=[arout.opt()])
        else:
            # AllGather both partials (no AllReduce cost multiplier on the
            # collective cores); the pairwise add happens locally on DVE.
            arout = dram.tile([2 * CW, D], BF16, tag="arout")
            nc.gpsimd.collective_compute(
                "AllGather", mybir.AluOpType.bypass,
                replica_groups=[[0, 1], [2, 3], [4, 5], [6, 7]],
                ins=[arin.opt()], outs=[arout.opt()])
        return arout

    def emit_tail_chunk(li, tcix, arout, gb1, gb2, ff_t, resTc_next, rT8_next):
        """AR result -> LN1 -> FFN -> LN2 -> resTc_next[tcix] (or output DMA).
        For the last layer the collective was a ReduceScatter: each core owns
        128 of the 256 chunk rows; the host reassembles."""
        ntl = 1 if li == n_layers - 1 else TPC
        xc = accb_pool.tile([128, TPC, D], BF16, tag="accb", name="xcur")
        if li == n_layers - 1:
            nc.gpsimd.dma_start(xc[:, 0:ntl, :],
                                arout[:].rearrange("(a p) d -> p a d", p=128))
        else:
            xg = wbig_pool.tile([128, 2, TPC, D], BF16, tag="wbig", name="xg")
            nc.gpsimd.dma_start(xg[:],
                                arout[:].rearrange("(g a p) d -> p g a d", p=128, g=2))
            nc.vector.tensor_add(xc[:, 0:ntl, :], xg[:, 0, :, :], xg[:, 1, :, :])
        xcur = [xc[:, tl, :] for tl in range(ntl)]
        layernorm(xcur, gb1)

        lnT = []
        for dc in range(DC):
            pp = ps_tp.tile([128, CW], BF16, tag="tp", name="tpps")
            for tl in range(ntl):
                nc.tensor.transpose(pp[:, tl * 128:(tl + 1) * 128],
                                    xc[:, tl, dc * 128:(dc + 1) * 128],
                                    identb[:])
            t = pt_pool.tile([128, 512], BF16, tag="pts", name="lnT")
            nc.scalar.copy(out=t[:, :ntl * 128], in_=pp[:, :ntl * 128])
            lnT.append(t)

        xmid = [xtd_pool.tile([128, D], F32, tag="xtd", name=f"xmid{tl}") for tl in range(ntl)]
        xmid_ap = [t[:] for t in xmid]
        for tl in range(ntl):
            for (n0, nw) in NCH:
                pm = mm_tile()
                for dc in range(DC):
                    nc.tensor.matmul(pm[:, :nw], lnT[dc][:, tl * 128:(tl + 1) * 128],
                                     ff_t[:, dc, n0:n0 + nw],
                                     start=(dc == 0), stop=(dc == DC - 1))
                nc.vector.tensor_copy(out=xmid[tl][:, n0:n0 + nw], in_=pm[:, :nw])

        layernorm(xmid_ap, gb2)

        if li < n_layers - 1:
            for dc in range(DC):
                pp = ps_tp.tile([128, CW], F32, tag="tp", name="tpps")
                for tl in range(TPC):
                    nc.tensor.transpose(pp[:, tl * 128:(tl + 1) * 128],
                                        xmid[tl][:, dc * 128:(dc + 1) * 128],
                                        ident[:])
                rt = rT_pool.tile([128, CW], BF16, tag="rT")
                nc.scalar.copy(out=rt[:], in_=pp[:])
                resTc_next[tcix][dc] = rt
                if dc % 2 == 0:
                    rT8_next[tcix][dc // 2] = rT8_pool.tile([128, 2, CW], F8, tag="rT8", name="rt8n")
                nc.vector.tensor_scalar_mul(rT8_next[tcix][dc // 2][:, dc % 2, :],
                                            pp[:], a_sc(li + 1))
        else:
            nc.sync.dma_start(out_d[tcix * 128:(tcix + 1) * 128, :], xmid[0][:])

    # ---- layers --------------------------------------------------------
    # prologue: layer-0 chunk-0 KV + QT prefetch (resTc from embeddings)
    cur_kt, cur_v = {}, {}
    qt_pre = {}
    for h in range(HPC):
        emit_kv_half(0, 0, h, rT8c, resTc, cur_kt, cur_v)
        if h < QT_AHEAD:
            qt_pre[h] = emit_qt(0, 0, h, rT8c)

    for li in range(n_layers):
        gb1 = load_ln_gb(li, "ln1")
        gb2 = load_ln_gb(li, "ln2")

        # A: KV s-half 1 (skewed) + chunk-0 score chains
        acc0 = [xtd_pool.tile([128, D], F32, tag="xtd", name=f"acc{tl}") for tl in range(TPC)]
        accb0 = accb_pool.tile([128, TPC, D], BF16, tag="accb", name="accb0")
        emit_kv_half(li, 1, 0, rT8c, resTc, cur_kt, cur_v)
        pend = None
        for h in range(HPC):
            qt_sb = qt_pre.pop(h) if h in qt_pre else emit_qt(li, 0, h, rT8c)
            if h + 1 < HPC:
                emit_kv_half(li, 1, h + 1, rT8c, resTc, cur_kt, cur_v)
            (pe_pair,) = emit_scores(li, 0, h, qt_sb, cur_kt)
            if pend is not None:
                ct_prev = emit_ctx(li, 0, pend[0], pend[1], cur_v)
            rec = emit_sums(pe_pair)
            if pend is not None:
                emit_out(li, 0, pend[0], ct_prev, pend[2], acc0, accb0)
            pend = (h, pe_pair, rec)
        ct_prev = emit_ctx(li, 0, pend[0], pend[1], cur_v)
        emit_out(li, 0, pend[0], ct_prev, pend[2], acc0, accb0)
        arout0 = emit_collective(li, accb0)

        # C: chunk-1 score chains (AR(c0) overlaps this)
        acc1 = [xtd_pool.tile([128, D], F32, tag="xtd", name=f"acc{tl}") for tl in range(TPC)]
        accb1 = accb_pool.tile([128, TPC, D], BF16, tag="accb", name="accb1")
        pend = None
        for h in range(HPC):
            qt_sb = emit_qt(li, 1, h, rT8c)
            if pend is not None:
                ct_prev = emit_ctx(li, 1, pend[0], pend[1], cur_v)
            (pe_pair,) = emit_scores(li, 1, h, qt_sb, cur_kt)
            if pend is not None:
                emit_out(li, 1, pend[0], ct_prev, pend[2], acc1, accb1)
            rec = emit_sums(pe_pair)
            pend = (h, pe_pair, rec)
        ct_prev = emit_ctx(li, 1, pend[0], pend[1], cur_v)
        emit_out(li, 1, pend[0], ct_prev, pend[2], acc1, accb1)

        ff_t = load_big(ff_d, li)

        # E: tail chunk 0 — emitted before the chunk-1 collective so its
        # gathered-load/add/LN serial chain runs during C on the free engines
        resTc_next = [[None] * DC for _ in range(NTC)]
        rT8_next = [[None] * JD for _ in range(NTC)]
        emit_tail_chunk(li, 0, arout0, gb1, gb2, ff_t, resTc_next, rT8_next)
        arout1 = emit_collective(li, accb1)

        # F: next layer's chunk-0 KV + QT prefetch (fills AR(c1) window)
        next_kt, next_v = {}, {}
        qt_pre = {}
        if li < n_layers - 1:
            for h in range(HPC):
                emit_kv_half(li + 1, 0, h, rT8_next, resTc_next, next_kt, next_v)
                if h < QT_AHEAD:
                    qt_pre[h] = emit_qt(li + 1, 0, h, rT8_next)

        # G: tail chunk 1
        emit_tail_chunk(li, 1, arout1, gb1, gb2, ff_t, resTc_next, rT8_next)

        resTc = resTc_next
        rT8c = rT8_next
        cur_kt, cur_v = next_kt, next_v


# ------------------------------------------------------------------------
# host side
# ------------------------------------------------------------------------
_CACHED = {}
_LAST_RES = None


def _get_nc(n_layers, flag_key, flags):
    key = (n_layers, flag_key)
    if key not in _CACHED:
        _CACHED[key] = build_nc(n_layers, flags)
    return _CACHED[key]


def _fp8(x, scale):
    return np.clip(x * scale, -240.0, 240.0).astype(E4)


def kernel(X, tok_w, tok_b, pos_w, pos_b, seg_w, seg_b,
           Wq, bq, Wk, bk, Wv, bv, Wo, bo,
           ln1_g, ln1_b, ffp_w, ffp_b, ln2_g, ln2_b, n_layers=L):
    global _LAST_RES
    f32 = np.float32
    X = np.asarray(X, dtype=np.int32)
    tok_w = np.asarray(tok_w, f32); pos_w = np.asarray(pos_w, f32); seg_w = np.asarray(seg_w, f32)
    Wq = np.asarray(Wq, f32); Wk = np.asarray(Wk, f32); Wv = np.asarray(Wv, f32)
    Wo = np.asarray(Wo, f32); ffp_w = np.asarray(ffp_w, f32)
    bq = np.asarray(bq, f32); bk = np.asarray(bk, f32); bv = np.asarray(bv, f32)
    bo = np.asarray(bo, f32); ffp_b = np.asarray(ffp_b, f32)
    ln1_g = np.asarray(ln1_g, f32); ln1_b = np.asarray(ln1_b, f32)
    ln2_g = np.asarray(ln2_g, f32); ln2_b = np.asarray(ln2_b, f32)
    tok_b = np.asarray(tok_b, f32); pos_b = np.asarray(pos_b, f32); seg_b = np.asarray(seg_b, f32)

    emb_bias = tok_b + pos_b + seg_b
    flags = {
        "emb_bias": bool(np.any(emb_bias)),
        "ln1": bool(np.any(ln1_g != 1) or np.any(ln1_b)),
        "ln2": bool(np.any(ln2_g != 1) or np.any(ln2_b)),
        "mask": bool(np.any(X[:, 0, :] == 0)),
    }
    assert not (np.any(bo) or np.any(ffp_b) or np.any(bq) or np.any(bk) or np.any(bv)), \
        "nonzero attention/ffn biases not implemented in this specialization"
    flag_key = tuple(sorted(flags.items()))
    nc = _get_nc(n_layers, flag_key, flags)

    tok_wb = tok_w.astype(BF)
    pos_wb = pos_w.astype(BF)
    seg_wb = seg_w.astype(BF)

    in_maps = []
    per_g = {}
    nl = n_layers
    for g in range(2):
        hsl = slice(g * HPC, (g + 1) * HPC)
        # [L, D, HK] per-group flattened weights
        wq_f = np.ascontiguousarray(Wq[:nl, :, hsl, :]).reshape(nl, D, HK)
        wk_f = np.ascontiguousarray(Wk[:nl, :, hsl, :]).reshape(nl, D, HK)
        wv_f = np.ascontiguousarray(Wv[:nl, :, hsl, :]).reshape(nl, D, HK)
        wo_f = np.ascontiguousarray(Wo[:nl, hsl, :, :]).reshape(nl, HK, D)
        # fp8 pair layout [L, h, p, j, i, 768]
        wq8 = np.ascontiguousarray(
            _fp8(wq_f, W_SC).reshape(nl, JD, 2, 128, HPC, KH).transpose(0, 4, 3, 1, 2, 5))
        wk8 = np.ascontiguousarray(
            _fp8(wk_f, W_SC).reshape(nl, JD, 2, 128, HPC, KH).transpose(0, 4, 3, 1, 2, 5))
        # bf16 per-head layouts
        wvh = np.ascontiguousarray(
            wv_f.astype(BF).reshape(nl, DC, 128, HPC, KH).transpose(0, 3, 2, 1, 4))
        woh = np.ascontiguousarray(
            wo_f.astype(BF).reshape(nl, HPC, DC, 128, D).transpose(0, 1, 3, 2, 4))
        per_g[g] = {"wq8": wq8, "wk8": wk8, "wv": wvh, "wo": woh}
    ffh = np.ascontiguousarray(
        ffp_w[:nl].astype(BF).reshape(nl, DC, 128, D).transpose(0, 2, 1, 3))

    for c in range(NCORES):
        b, g = c // 2, c % 2
        m = {
            "xids": np.ascontiguousarray(X[b]),
            "tok_w": tok_wb, "pos_w": pos_wb, "seg_w": seg_wb,
            "ff": ffh,
            **per_g[g],
        }
        if flags["emb_bias"]:
            m["emb_bias"] = emb_bias
        if flags["ln1"]:
            m["ln1_g"] = np.ascontiguousarray(ln1_g[:nl])
            m["ln1_b"] = np.ascontiguousarray(ln1_b[:nl])
        if flags["ln2"]:
            m["ln2_g"] = np.ascontiguousarray(ln2_g[:nl])
            m["ln2_b"] = np.ascontiguousarray(ln2_b[:nl])
        if flags["mask"]:
            m["maskneg"] = np.where(X[b, 0, :] == 0, -1e9 * QK_SC * QK_SC, 0.0).astype(f32)
        in_maps.append(m)

    res = bass_utils.run_bass_kernel_spmd(nc, in_maps, core_ids=list(range(NCORES)))
    _LAST_RES = res
    out = np.empty((B, S, D), np.float32)
    for b in range(B):
        o0 = res.results[2 * b]["out"]      # rank-0 shards: rows 0:128 / 256:384
        o1 = res.results[2 * b + 1]["out"]  # rank-1 shards: rows 128:256 / 384:512
        out[b, 0:128] = o0[0:128]
        out[b, 128:256] = o1[0:128]
        out[b, 256:384] = o0[128:256]
        out[b, 384:512] = o1[128:256]
    return out


# revision 7
# speedup vs baseline: 1.0635x; 1.0023x over previous
"""Trainium2 Bass kernel for nn_JslBERT — v2: fp8 DoubleRow Q/K/scores.

Sharding: 8 cores = 4 batch x 2 head-groups (6 heads each). Per layer,
attention-output partials are pairwise AllReduced in bf16; LN+FFN run
redundantly on both cores of a pair.

v2 changes vs baseline:
 - Q-proj, K-proj and QK^T scores run in fp8(e4m3) with perf_mode=DoubleRow
   (contraction pairs of 128-chunks per instruction). Static power-of-2
   scaling: weights x1024, residual x16 (x512 in layer 0), Q^T/K^T stored
   x32; descale folded into the PSUM drains and the exp() scale.
   V/ctx/out-proj/FFN stay bf16 (precision-critical path; fp8 there blows
   the 2e-2 gate).
 - Weight DMAs consolidated: one DMA per (tensor, head[, chunk]) instead of
   one per 128-row d-chunk (HWDGE instruction-count was near saturation).
"""
import os
import numpy as np
import ml_dtypes

import concourse.bass as bass
import concourse.bacc as bacc
import concourse.tile as tile
import concourse.bass_utils as bass_utils
from concourse import mybir
from concourse.masks import make_identity

# Model dims (hardcoded per problem spec)
B, S, L, D, H, V, PMAX = 4, 512, 4, 768, 12, 32000, 512
EPS = 1e-3
NCORES = 8
HPC = H // 2          # heads per core
KH = D                # head dim (768)
HK = HPC * KH         # 4608 flattened head dims per core
SCALE = 1.0 / float(np.sqrt(D))

F32 = mybir.dt.float32
BF16 = mybir.dt.bfloat16
F8 = mybir.dt.float8e4
I32 = mybir.dt.int32
DR = mybir.MatmulPerfMode.DoubleRow

TT = S // 128         # 4 t-tiles total
DC = D // 128         # 6 d-chunks
JD = DC // 2          # 3 d-chunk pairs (DoubleRow)
NCH = [(0, 512), (512, 256)]  # free-dim chunks for width-768 outputs
NTC = 2               # t-chunks per sequence
TPC = TT // NTC       # 128-tiles per chunk (2)
CW = S // NTC         # chunk width (256)

# fp8 static scales
W_SC = 1024.0                     # wq/wk weights
QK_SC = 32.0                      # stored Q^T / K^T
EXP_SCALE = SCALE / (QK_SC * QK_SC)


def a_sc(li):                     # residual-stream fp8 scale
    return 512.0 if li == 0 else 16.0


def qk_drain(li):                 # PSUM -> qt/kt fp8 multiplier
    return QK_SC / (a_sc(li) * W_SC)


BF = np.dtype(ml_dtypes.bfloat16)
E4 = np.dtype(ml_dtypes.float8_e4m3)


def build_nc(n_layers=L, flags=None):
    """Build the Bass graph. flags: dict of which optional inputs exist."""
    flags = flags or {}
    nc = bacc.Bacc("TRN2", target_bir_lowering=False, debug=False,
                   num_devices=NCORES)

    xids_d = nc.dram_tensor("xids", [3, S], I32, kind="ExternalInput").ap()
    tokw_d = nc.dram_tensor("tok_w", [V, D], BF16, kind="ExternalInput").ap()
    posw_d = nc.dram_tensor("pos_w", [PMAX, D], BF16, kind="ExternalInput").ap()
    segw_d = nc.dram_tensor("seg_w", [2, D], BF16, kind="ExternalInput").ap()
    # fp8 pair layouts: [L, h, p, j, i, 768] with slot i = d-chunk 2j+i
    wq_d = nc.dram_tensor("wq8", [n_layers, HPC, 128, JD, 2, KH], F8, kind="ExternalInput").ap()
    wk_d = nc.dram_tensor("wk8", [n_layers, HPC, 128, JD, 2, KH], F8, kind="ExternalInput").ap()
    # bf16 per-head layouts: [L, h, p, dc|kc, 768]
    wv_d = nc.dram_tensor("wv", [n_layers, HPC, 128, DC, KH], BF16, kind="ExternalInput").ap()
    wo_d = nc.dram_tensor("wo", [n_layers, HPC, 128, DC, D], BF16, kind="ExternalInput").ap()
    ff_d = nc.dram_tensor("ff", [n_layers, 128, DC, D], BF16, kind="ExternalInput").ap()
    out_d = nc.dram_tensor("out", [S, D], F32, kind="ExternalOutput").ap()

    opt = {}
    if flags.get("emb_bias"):
        opt["emb_bias"] = nc.dram_tensor("emb_bias", [D], F32, kind="ExternalInput").ap()
    for nm in ("ln1", "ln2"):
        if flags.get(nm):
            opt[nm + "_g"] = nc.dram_tensor(nm + "_g", [n_layers, D], F32, kind="ExternalInput").ap()
            opt[nm + "_b"] = nc.dram_tensor(nm + "_b", [n_layers, D], F32, kind="ExternalInput").ap()
    if flags.get("mask"):
        opt["maskneg"] = nc.dram_tensor("maskneg", [S], F32, kind="ExternalInput").ap()

    with tile.TileContext(nc) as tc:
        import contextlib
        with contextlib.ExitStack() as ctx:
            _build_body(ctx, tc, n_layers, flags, xids_d, tokw_d, posw_d, segw_d,
                        wq_d, wk_d, wv_d, wo_d, ff_d, out_d, opt)
    nc.compile()
    return nc


def _build_body(ctx, tc, n_layers, flags, xids_d, tokw_d, posw_d, segw_d,
                wq_d, wk_d, wv_d, wo_d, ff_d, out_d, opt):
    nc = tc.nc

    const = ctx.enter_context(tc.tile_pool(name="const", bufs=1))
    wq8_pool = ctx.enter_context(tc.tile_pool(name="wq8", bufs=2))
    wk8_pool = ctx.enter_context(tc.tile_pool(name="wk8", bufs=2))
    wbig_pool = ctx.enter_context(tc.tile_pool(name="wbig", bufs=3))   # wv/wo
    ff_pool = ctx.enter_context(tc.tile_pool(name="ffp", bufs=1))
    rT8_pool = ctx.enter_context(tc.tile_pool(name="rT8", bufs=8))
    rT_pool = ctx.enter_context(tc.tile_pool(name="rT", bufs=13))
    kt_pool = ctx.enter_context(tc.tile_pool(name="ktp", bufs=18))
    v_pool = ctx.enter_context(tc.tile_pool(name="vp", bufs=24))
    qt_pool = ctx.enter_context(tc.tile_pool(name="qtp", bufs=19))
    pe_pool = ctx.enter_context(tc.tile_pool(name="pep", bufs=5))
    pt_pool = ctx.enter_context(tc.tile_pool(name="ptp", bufs=8))
    ct_pool = ctx.enter_context(tc.tile_pool(name="ctp", bufs=5))
    xtd_pool = ctx.enter_context(tc.tile_pool(name="xtd", bufs=6))
    accb_pool = ctx.enter_context(tc.tile_pool(name="accb", bufs=4))
    sm_pool = ctx.enter_context(tc.tile_pool(name="sm", bufs=8))
    ps_mm = ctx.enter_context(tc.tile_pool(name="psmm", bufs=6, space="PSUM"))
    ps_tp = ctx.enter_context(tc.tile_pool(name="pstp", bufs=2, space="PSUM"))
    dram = ctx.enter_context(tc.tile_pool(name="dram", bufs=2, space="DRAM"))

    ident = const.tile([128, 128], F32)
    make_identity(nc, ident[:])
    identb = const.tile([128, 128], BF16)
    make_identity(nc, identb[:])
    eps_t = const.tile([128, 1], F32)
    nc.vector.memset(eps_t[:], EPS)
    ones_t = const.tile([128, 1], BF16)
    nc.vector.memset(ones_t[:], 1.0)

    def mm_tile():
        return ps_mm.tile([128, 512], F32, tag="mm", name="mmps")

    # ---- weight loaders ------------------------------------------------
    def load_wq8(li, h):
        t = wq8_pool.tile([128, JD, 2, KH], F8, tag="wq8")
        nc.sync.dma_start(t[:], wq_d[li, h])
        return t

    def load_wk8(li, h):
        t = wk8_pool.tile([128, JD, 2, KH], F8, tag="wk8")
        nc.sync.dma_start(t[:], wk_d[li, h])
        return t

    def load_big(wd, li, h=None):
        pool = wbig_pool if h is not None else ff_pool
        t = pool.tile([128, DC, D], BF16, tag="wbig" if h is not None else "ff")
        nc.sync.dma_start(t[:], wd[li] if h is None else wd[li, h])
        return t

    # ---- embeddings ----------------------------------------------------
    idx = const.tile([128, 3, TT], I32)
    nc.sync.dma_start(idx[:], xids_d.rearrange("k (j p) -> p k j", p=128))

    emb_bias_ap = None
    if "emb_bias" in opt:
        eb = const.tile([128, DC], F32)
        nc.sync.dma_start(eb[:], opt["emb_bias"].rearrange("(c p) -> p c", p=128))
        emb_bias_ap = [eb[:, c:c + 1] for c in range(DC)]

    # pos ids are structurally arange(S) (built that way in the model), so the
    # pos "lookup" is a direct row DMA; tok/seg stay data-dependent gathers.
    tok_t = wbig_pool.tile([128, DC, D], BF16, tag="wbig", name="tokt")
    pos_t = wbig_pool.tile([128, DC, D], BF16, tag="wbig", name="post")
    seg_t = wbig_pool.tile([128, DC, D], BF16, tag="wbig", name="segt")
    nc.sync.dma_start(pos_t[:, 0:TT, :],
                      posw_d.rearrange("(a p) d -> p a d", p=128))
    for tm in range(TT):
        nc.gpsimd.indirect_dma_start(
            out=tok_t[:, tm, :], out_offset=None, in_=tokw_d[:],
            in_offset=bass.IndirectOffsetOnAxis(ap=idx[:, 0, tm:tm + 1], axis=0))
        nc.gpsimd.indirect_dma_start(
            out=seg_t[:, tm, :], out_offset=None, in_=segw_d[:],
            in_offset=bass.IndirectOffsetOnAxis(ap=idx[:, 2, tm:tm + 1], axis=0))
    for tm in range(TT):
        nc.vector.tensor_add(tok_t[:, tm, :], tok_t[:, tm, :], pos_t[:, tm, :])
        nc.vector.tensor_add(tok_t[:, tm, :], tok_t[:, tm, :], seg_t[:, tm, :])

    # resTc[tcix][dc]: [128 d, 256 t] bf16 (V-proj stationary)
    # rT8c[tcix][j]:  [128 d, 2, 256 t] fp8 pairs (Q/K moving operand)
    resTc = [[None] * DC for _ in range(NTC)]
    rT8c = [[None] * JD for _ in range(NTC)]
    for tcix in range(NTC):
        for dc in range(DC):
            pp = ps_tp.tile([128, CW], BF16, tag="tp", name="tpps")
            for tl in range(TPC):
                nc.tensor.transpose(pp[:, tl * 128:(tl + 1) * 128],
                                    tok_t[:, tcix * TPC + tl, dc * 128:(dc + 1) * 128],
                                    identb[:])
            rt = rT_pool.tile([128, CW], BF16, tag="rT")
            if emb_bias_ap is not None:
                nc.vector.tensor_scalar_add(rt[:], pp[:], emb_bias_ap[dc])
            else:
                nc.vector.tensor_copy(out=rt[:], in_=pp[:])
            resTc[tcix][dc] = rt
            if dc % 2 == 0:
                rT8c[tcix][dc // 2] = rT8_pool.tile([128, 2, CW], F8, tag="rT8", name="rt8e")
            if emb_bias_ap is not None:
                nc.vector.tensor_scalar(out=rT8c[tcix][dc // 2][:, dc % 2, :],
                                        in0=pp[:], scalar1=emb_bias_ap[dc],
                                        scalar2=a_sc(0),
                                        op0=mybir.AluOpType.add,
                                        op1=mybir.AluOpType.mult)
            else:
                nc.vector.tensor_scalar_mul(rT8c[tcix][dc // 2][:, dc % 2, :],
                                            pp[:], a_sc(0))

    mask_ap = None
    if "maskneg" in opt:
        mk = const.tile([128, TT], F32)
        nc.sync.dma_start(mk[:], opt["maskneg"].rearrange("(a p) -> p a", p=128))
        mask_ap = mk

    # ---- per-layer helpers --------------------------------------------
    QT_AHEAD = 6  # heads whose chunk-0 QT is prefetched at the end of the previous layer

    def load_ln_gb(li, nm):
        if nm + "_g" not in opt:
            return None
        gb = const.tile([128, 2, D], F32, tag=f"lngb{nm}{li}")
        nc.sync.dma_start(gb[:, 0, :], opt[nm + "_g"][li].partition_broadcast(128))
        nc.sync.dma_start(gb[:, 1, :], opt[nm + "_b"][li].partition_broadcast(128))
        return gb

    def layernorm(aps, gb):
        for x in aps:
            stats = sm_pool.tile([128, 3, 6], F32, tag="bnst")
            mv = sm_pool.tile([128, 2], F32, tag="bnmv")
            xg = x.rearrange("p (a c) -> p a c", a=3)
            for a in range(3):
                nc.vector.bn_stats(out=stats[:, a, :], in_=xg[:, a, :])
            nc.vector.bn_aggr(out=mv[:], in_=stats[:])
            rstd = sm_pool.tile([128, 1], F32, tag="rstd")
            nc.scalar.activation(out=rstd[:], in_=mv[:, 1:2],
                                 func=mybir.ActivationFunctionType.Sqrt,
                                 bias=eps_t[:], scale=1.0)
            nc.vector.reciprocal(rstd[:], rstd[:])
            nc.vector.tensor_scalar(out=x, in0=x, scalar1=mv[:, 0:1],
                                    scalar2=rstd[:],
                                    op0=mybir.AluOpType.subtract,
                                    op1=mybir.AluOpType.mult)
            if gb is not None:
                nc.vector.tensor_mul(x, x, gb[:, 0, :])
                nc.vector.tensor_add(x, x, gb[:, 1, :])

    def emit_kv_half(li, sc, h, rT8_l, resTc_l, kt_all, v_all):
        """KT s-half (fp8 DoubleRow) + V s-half (bf16) for one head.
        kt[(h,j)]: [128 k, 2, 512 s] fp8 pairs, v[(h,sm)]: [128 s, 768 k] bf16."""
        wk8 = load_wk8(li, h)
        wv_t = load_big(wv_d, li, h)
        dr_sc = qk_drain(li)
        for m2 in range(JD):
            pm = mm_tile()
            for half in range(2):
                m = 2 * m2 + half
                for j in range(JD):
                    nc.tensor.matmul(pm[:, half * CW:half * CW + CW],
                                     wk8[:, j, :, m * 128:(m + 1) * 128],
                                     rT8_l[sc][j][:],
                                     start=(j == 0), stop=(j == JD - 1),
                                     perf_mode=DR)
            if sc == 0:
                kt_all[(h, m2)] = kt_pool.tile([128, 2, S], F8, tag="kt",
                                               name=f"kt{h}_{m2}")
            pmv = pm[:].rearrange("p (i c) -> p i c", i=2)
            if m2 % 2 == 0:
                nc.scalar.mul(kt_all[(h, m2)][:, :, sc * CW:(sc + 1) * CW],
                              pmv, dr_sc)
            else:
                nc.vector.tensor_scalar_mul(
                    kt_all[(h, m2)][:, :, sc * CW:(sc + 1) * CW],
                    pmv, dr_sc)
        for tl in range(TPC):
            sm = sc * TPC + tl
            vt = v_pool.tile([128, D], BF16, tag="v")
            v_all[(h, sm)] = vt
            for (n0, nw) in NCH:
                pm = mm_tile()
                for dc in range(DC):
                    nc.tensor.matmul(pm[:, :nw],
                                     resTc_l[sc][dc][:, tl * 128:(tl + 1) * 128],
                                     wv_t[:, dc, n0:n0 + nw],
                                     start=(dc == 0), stop=(dc == DC - 1))
                nc.scalar.copy(out=vt[:, n0:n0 + nw], in_=pm[:, :nw])

    def emit_qt(li, tcix, h, rT8_l):
        """QT for one head/chunk via fp8 DoubleRow, packed 2 m's per PSUM bank.
        Returns 3 tiles [128, 512] fp8: tile j = m (2j, 2j+1) x 256 t."""
        wq8 = load_wq8(li, h)
        dr_sc = qk_drain(li)
        qt_sb = []
        for jo in range(JD):
            pm = mm_tile()
            for half in range(2):
                m = 2 * jo + half
                for j in range(JD):
                    nc.tensor.matmul(pm[:, half * CW:half * CW + CW],
                                     wq8[:, j, :, m * 128:(m + 1) * 128],
                                     rT8_l[tcix][j][:],
                                     start=(j == 0), stop=(j == JD - 1),
                                     perf_mode=DR)
            ot = qt_pool.tile([128, 512], F8, tag="qt")
            if jo % 2 == 0:
                nc.vector.tensor_scalar_mul(ot[:], pm[:], dr_sc)
            else:
                nc.scalar.mul(ot[:], pm[:], dr_sc)
            qt_sb.append(ot)
        return qt_sb

    def emit_scores(li, tcix, h, qt_sb, kt_all):
        """Transposed scores: peT[s, t] = exp(K Q^T) via fp8 DoubleRow, plus
        per-t 1/sum (applied later at the out-proj drain). Removes the whole
        P-transpose stage. pe_pair[smp] is [128 s, (2 sm-half)(256 t)] bf16 —
        the same layout the old pt_sb had, so ctx consumes it unchanged."""
        pe_pair = []
        for smp in range(2):
            pm = mm_tile()
            for half in range(2):
                sm = 2 * smp + half
                for j in range(JD):
                    qv = qt_sb[j][:].rearrange("p (i c) -> p i c", i=2)
                    nc.tensor.matmul(pm[:, half * CW:half * CW + CW],
                                     kt_all[(h, j)][:, :, sm * 128:(sm + 1) * 128],
                                     qv,
                                     start=(j == 0), stop=(j == JD - 1),
                                     perf_mode=DR)
                if mask_ap is not None:
                    nc.vector.tensor_scalar_add(pm[:, half * CW:half * CW + CW],
                                                pm[:, half * CW:half * CW + CW],
                                                mask_ap[:, sm:sm + 1])
            pe = pe_pool.tile([128, S], BF16, tag="pe")
            nc.scalar.activation(out=pe[:], in_=pm[:],
                                 func=mybir.ActivationFunctionType.Exp,
                                 scale=EXP_SCALE)
            pe_pair.append(pe)
        return (pe_pair,)

    def emit_sums(pe_pair):
        """per-t softmax denominators from peT via ones-matmuls (deferred so
        the PE queue isn't head-of-line blocked on the exp while dense work
        from the previous head is available)."""
        sp = ps_tp.tile([128, 2], F32, tag="tp", name="sumsps")
        for tb in range(TPC):
            k = 0
            for smp in range(2):
                for half in range(2):
                    nc.tensor.matmul(sp[:, tb:tb + 1],
                                     pe_pair[smp][:, half * CW + tb * 128:
                                                  half * CW + (tb + 1) * 128],
                                     ones_t[:], start=(k == 0), stop=(k == 3))
                    k += 1
        rec = sm_pool.tile([128, 2], F32, tag="rec")
        nc.vector.reciprocal(rec[:], sp[:])
        return rec

    def emit_ctx(li, tcix, h, pe_pair, v_all):
        """ctxT from unnormalized peT."""
        # ctxT packed: tile j holds km (2j | 2j+1) x 256 t (unnormalized)
        ct_sb = []
        for j in range(DC // 2):
            pm = mm_tile()
            for half in range(2):
                km = 2 * j + half
                for sm in range(TT):
                    nc.tensor.matmul(pm[:, half * CW:half * CW + CW],
                                     v_all[(h, sm)][:, km * 128:(km + 1) * 128],
                                     pe_pair[sm // 2][:, (sm % 2) * CW:(sm % 2) * CW + CW],
                                     start=(sm == 0), stop=(sm == TT - 1))
            ot = ct_pool.tile([128, 512], BF16, tag="ct")
            nc.vector.tensor_copy(out=ot[:], in_=pm[:])
            ct_sb.append(ot)
        return ct_sb

    def emit_out(li, tcix, h, ct_sb, rec, acc, accb):
        """out-proj partial with 1/sum folded into the accumulate as a
        per-t-partition scalar."""
        wo_t = load_big(wo_d, li, h)
        for tl in range(TPC):
            for (n0, nw) in NCH:
                pm = mm_tile()
                for kc in range(DC):
                    nc.tensor.matmul(pm[:, :nw],
                                     ct_sb[kc // 2][:, (kc % 2) * CW + tl * 128:
                                                    (kc % 2) * CW + (tl + 1) * 128],
                                     wo_t[:, kc, n0:n0 + nw],
                                     start=(kc == 0), stop=(kc == DC - 1))
                if h == 0:
                    nc.vector.tensor_scalar_mul(acc[tl][:, n0:n0 + nw],
                                                pm[:, :nw], rec[:, tl:tl + 1])
                elif h < HPC - 1:
                    nc.vector.scalar_tensor_tensor(
                        out=acc[tl][:, n0:n0 + nw], in0=pm[:, :nw],
                        scalar=rec[:, tl:tl + 1], in1=acc[tl][:, n0:n0 + nw],
                        op0=mybir.AluOpType.mult, op1=mybir.AluOpType.add)
                else:
                    nc.vector.scalar_tensor_tensor(
                        out=accb[:, tl, n0:n0 + nw], in0=pm[:, :nw],
                        scalar=rec[:, tl:tl + 1], in1=acc[tl][:, n0:n0 + nw],
                        op0=mybir.AluOpType.mult, op1=mybir.AluOpType.add)

    def emit_collective(li, accb):
        arin = dram.tile([CW, D], BF16, tag="arin")
        last = li == n_layers - 1
        nc.sync.dma_start(arin[:].rearrange("(a p) d -> p a d", p=128),
                          accb[:, 0:TPC, :])
        if last:
            arout = dram.tile([128, D], BF16, tag="arout2")
            nc.gpsimd.collective_compute(
                "ReduceScatter", mybir.AluOpType.add,
                replica_groups=[[0, 1], [2, 3], [4, 5], [6, 7]],
                ins=[arin.opt()], outs=[arout.opt()])
        else:
            # AllGather both partials (no AllReduce cost multiplier on the
            # collective cores); the pairwise add happens locally on DVE.
            arout = dram.tile([2 * CW, D], BF16, tag="arout")
            nc.gpsimd.collective_compute(
                "AllGather", mybir.AluOpType.bypass,
                replica_groups=[[0, 1], [2, 3], [4, 5], [6, 7]],
                ins=[arin.opt()], outs=[arout.opt()])
        return arout

    def emit_tail_chunk(li, tcix, arout, gb1, gb2, ff_t, resTc_next, rT8_next):
        """AR result -> LN1 -> FFN -> LN2 -> resTc_next[tcix] (or output DMA).
        For the last layer the collective was a ReduceScatter: each core owns
        128 of the 256 chunk rows; the host reassembles."""
        ntl = 1 if li == n_layers - 1 else TPC
        xc = accb_pool.tile([128, TPC, D], BF16, tag="accb", name="xcur")
        if li == n_layers - 1:
            nc.gpsimd.dma_start(xc[:, 0:ntl, :],
                                arout[:].rearrange("(a p) d -> p a d", p=128))
        else:
            xg = wbig_pool.tile([128, 2, TPC, D], BF16, tag="wbig", name="xg")
            nc.gpsimd.dma_start(xg[:],
                                arout[:].rearrange("(g a p) d -> p g a d", p=128, g=2))
            nc.vector.tensor_add(xc[:, 0:ntl, :], xg[:, 0, :, :], xg[:, 1, :, :])
        xcur = [xc[:, tl, :] for tl in range(ntl)]
        layernorm(xcur, gb1)

        lnT = []
        for dc in range(DC):
            pp = ps_tp.tile([128, CW], BF16, tag="tp", name="tpps")
            for tl in range(ntl):
                nc.tensor.transpose(pp[:, tl * 128:(tl + 1) * 128],
                                    xc[:, tl, dc * 128:(dc + 1) * 128],
                                    identb[:])
            t = pt_pool.tile([128, 512], BF16, tag="pts", name="lnT")
            nc.scalar.copy(out=t[:, :ntl * 128], in_=pp[:, :ntl * 128])
            lnT.append(t)

        xmid = [xtd_pool.tile([128, D], F32, tag="xtd", name=f"xmid{tl}") for tl in range(ntl)]
        xmid_ap = [t[:] for t in xmid]
        for tl in range(ntl):
            for (n0, nw) in NCH:
                pm = mm_tile()
                for dc in range(DC):
                    nc.tensor.matmul(pm[:, :nw], lnT[dc][:, tl * 128:(tl + 1) * 128],
                                     ff_t[:, dc, n0:n0 + nw],
                                     start=(dc == 0), stop=(dc == DC - 1))
                nc.vector.tensor_copy(out=xmid[tl][:, n0:n0 + nw], in_=pm[:, :nw])

        layernorm(xmid_ap, gb2)

        if li < n_layers - 1:
            for dc in range(DC):
                pp = ps_tp.tile([128, CW], F32, tag="tp", name="tpps")
                for tl in range(TPC):
                    nc.tensor.transpose(pp[:, tl * 128:(tl + 1) * 128],
                                        xmid[tl][:, dc * 128:(dc + 1) * 128],
                                        ident[:])
                rt = rT_pool.tile([128, CW], BF16, tag="rT")
                nc.scalar.copy(out=rt[:], in_=pp[:])
                resTc_next[tcix][dc] = rt
                if dc % 2 == 0:
                    rT8_next[tcix][dc // 2] = rT8_pool.tile([128, 2, CW], F8, tag="rT8", name="rt8n")
                nc.vector.tensor_scalar_mul(rT8_next[tcix][dc // 2][:, dc % 2, :],
                                            pp[:], a_sc(li + 1))
        else:
            nc.sync.dma_start(out_d[tcix * 128:(tcix + 1) * 128, :], xmid[0][:])

    # ---- layers --------------------------------------------------------
    # prologue: layer-0 chunk-0 KV + QT prefetch (resTc from embeddings)
    cur_kt, cur_v = {}, {}
    qt_pre = {}
    for h in range(HPC):
        emit_kv_half(0, 0, h, rT8c, resTc, cur_kt, cur_v)
        if h < QT_AHEAD:
            qt_pre[h] = emit_qt(0, 0, h, rT8c)

    for li in range(n_layers):
        gb1 = load_ln_gb(li, "ln1")
        gb2 = load_ln_gb(li, "ln2")

        # A: KV s-half 1 (skewed) + chunk-0 score chains
        acc0 = [xtd_pool.tile([128, D], F32, tag="xtd", name=f"acc{tl}") for tl in range(TPC)]
        accb0 = accb_pool.tile([128, TPC, D], BF16, tag="accb", name="accb0")
        emit_kv_half(li, 1, 0, rT8c, resTc, cur_kt, cur_v)
        pend = None
        for h in range(HPC):
            qt_sb = qt_pre.pop(h) if h in qt_pre else emit_qt(li, 0, h, rT8c)
            if h + 1 < HPC:
                emit_kv_half(li, 1, h + 1, rT8c, resTc, cur_kt, cur_v)
            (pe_pair,) = emit_scores(li, 0, h, qt_sb, cur_kt)
            if pend is not None:
                ct_prev = emit_ctx(li, 0, pend[0], pend[1], cur_v)
            rec = emit_sums(pe_pair)
            if pend is not None:
                emit_out(li, 0, pend[0], ct_prev, pend[2], acc0, accb0)
            pend = (h, pe_pair, rec)
        ct_prev = emit_ctx(li, 0, pend[0], pend[1], cur_v)
        emit_out(li, 0, pend[0], ct_prev, pend[2], acc0, accb0)
        arout0 = emit_collective(li, accb0)

        # C: chunk-1 score chains (AR(c0) overlaps this)
        acc1 = [xtd_pool.tile([128, D], F32, tag="xtd", name=f"acc{tl}") for tl in range(TPC)]
        accb1 = accb_pool.tile([128, TPC, D], BF16, tag="accb", name="accb1")
        pend = None
        for h in range(HPC):
            qt_sb = emit_qt(li, 1, h, rT8c)
            if pend is not None:
                ct_prev = emit_ctx(li, 1, pend[0], pend[1], cur_v)
            (pe_pair,) = emit_scores(li, 1, h, qt_sb, cur_kt)
            if pend is not None:
                emit_out(li, 1, pend[0], ct_prev, pend[2], acc1, accb1)
            rec = emit_sums(pe_pair)
            pend = (h, pe_pair, rec)
        ct_prev = emit_ctx(li, 1, pend[0], pend[1], cur_v)
        emit_out(li, 1, pend[0], ct_prev, pend[2], acc1, accb1)

        ff_t = load_big(ff_d, li)

        # E: tail chunk 0 — emitted before the chunk-1 collective so its
        # gathered-load/add/LN serial chain runs during C on the free engines
        resTc_next = [[None] * DC for _ in range(NTC)]
        rT8_next = [[None] * JD for _ in range(NTC)]
        emit_tail_chunk(li, 0, arout0, gb1, gb2, ff_t, resTc_next, rT8_next)
        arout1 = emit_collective(li, accb1)

        # F: next layer's chunk-0 KV + QT prefetch (fills AR(c1) window)
        next_kt, next_v = {}, {}
        qt_pre = {}
        if li < n_layers - 1:
            for h in range(HPC):
                emit_kv_half(li + 1, 0, h, rT8_next, resTc_next, next_kt, next_v)
                if h < QT_AHEAD:
                    qt_pre[h] = emit_qt(li + 1, 0, h, rT8_next)

        # G: tail chunk 1
        emit_tail_chunk(li, 1, arout1, gb1, gb2, ff_t, resTc_next, rT8_next)

        resTc = resTc_next
        rT8c = rT8_next
        cur_kt, cur_v = next_kt, next_v


# ------------------------------------------------------------------------
# host side
# ------------------------------------------------------------------------
_CACHED = {}
_LAST_RES = None


def _get_nc(n_layers, flag_key, flags):
    key = (n_layers, flag_key)
    if key not in _CACHED:
        _CACHED[key] = build_nc(n_layers, flags)
    return _CACHED[key]


def _fp8(x, scale):
    return np.clip(x * scale, -240.0, 240.0).astype(E4)


def kernel(X, tok_w, tok_b, pos_w, pos_b, seg_w, seg_b,
           Wq, bq, Wk, bk, Wv, bv, Wo, bo,
           ln1_g, ln1_b, ffp_w, ffp_b, ln2_g, ln2_b, n_layers=L):
    global _LAST_RES
    f32 = np.float32
    X = np.asarray(X, dtype=np.int32)
    tok_w = np.asarray(tok_w, f32); pos_w = np.asarray(pos_w, f32); seg_w = np.asarray(seg_w, f32)
    Wq = np.asarray(Wq, f32); Wk = np.asarray(Wk, f32); Wv = np.asarray(Wv, f32)
    Wo = np.asarray(Wo, f32); ffp_w = np.asarray(ffp_w, f32)
    bq = np.asarray(bq, f32); bk = np.asarray(bk, f32); bv = np.asarray(bv, f32)
    bo = np.asarray(bo, f32); ffp_b = np.asarray(ffp_b, f32)
    ln1_g = np.asarray(ln1_g, f32); ln1_b = np.asarray(ln1_b, f32)
    ln2_g = np.asarray(ln2_g, f32); ln2_b = np.asarray(ln2_b, f32)
    tok_b = np.asarray(tok_b, f32); pos_b = np.asarray(pos_b, f32); seg_b = np.asarray(seg_b, f32)

    emb_bias = tok_b + pos_b + seg_b
    flags = {
        "emb_bias": bool(np.any(emb_bias)),
        "ln1": bool(np.any(ln1_g != 1) or np.any(ln1_b)),
        "ln2": bool(np.any(ln2_g != 1) or np.any(ln2_b)),
        "mask": bool(np.any(X[:, 0, :] == 0)),
    }
    assert not (np.any(bo) or np.any(ffp_b) or np.any(bq) or np.any(bk) or np.any(bv)), \
        "nonzero attention/ffn biases not implemented in this specialization"
    flag_key = tuple(sorted(flags.items()))
    nc = _get_nc(n_layers, flag_key, flags)

    tok_wb = tok_w.astype(BF)
    pos_wb = pos_w.astype(BF)
    seg_wb = seg_w.astype(BF)

    in_maps = []
    per_g = {}
    nl = n_layers
    for g in range(2):
        hsl = slice(g * HPC, (g + 1) * HPC)
        # [L, D, HK] per-group flattened weights
        wq_f = np.ascontiguousarray(Wq[:nl, :, hsl, :]).reshape(nl, D, HK)
        wk_f = np.ascontiguousarray(Wk[:nl, :, hsl, :]).reshape(nl, D, HK)
        wv_f = np.ascontiguousarray(Wv[:nl, :, hsl, :]).reshape(nl, D, HK)
        wo_f = np.ascontiguousarray(Wo[:nl, hsl, :, :]).reshape(nl, HK, D)
        # fp8 pair layout [L, h, p, j, i, 768]
        wq8 = np.ascontiguousarray(
            _fp8(wq_f, W_SC).reshape(nl, JD, 2, 128, HPC, KH).transpose(0, 4, 3, 1, 2, 5))
        wk8 = np.ascontiguousarray(
            _fp8(wk_f, W_SC).reshape(nl, JD, 2, 128, HPC, KH).transpose(0, 4, 3, 1, 2, 5))
        # bf16 per-head layouts
        wvh = np.ascontiguousarray(
            wv_f.astype(BF).reshape(nl, DC, 128, HPC, KH).transpose(0, 3, 2, 1, 4))
        woh = np.ascontiguousarray(
            wo_f.astype(BF).reshape(nl, HPC, DC, 128, D).transpose(0, 1, 3, 2, 4))
        per_g[g] = {"wq8": wq8, "wk8": wk8, "wv": wvh, "wo": woh}
    ffh = np.ascontiguousarray(
        ffp_w[:nl].astype(BF).reshape(nl, DC, 128, D).transpose(0, 2, 1, 3))

    for c in range(NCORES):
        b, g = c // 2, c % 2
        m = {
            "xids": np.ascontiguousarray(X[b]),
            "tok_w": tok_wb, "pos_w": pos_wb, "seg_w": seg_wb,
            "ff": ffh,
            **per_g[g],
        }
        if flags["emb_bias"]:
            m["emb_bias"] = emb_bias
        if flags["ln1"]:
            m["ln1_g"] = np.ascontiguousarray(ln1_g[:nl])
            m["ln1_b"] = np.ascontiguousarray(ln1_b[:nl])
        if flags["ln2"]:
            m["ln2_g"] = np.ascontiguousarray(ln2_g[:nl])
            m["ln2_b"] = np.ascontiguousarray(ln2_b[:nl])
        if flags["mask"]:
            m["maskneg"] = np.where(X[b, 0, :] == 0, -1e9 * QK_SC * QK_SC, 0.0).astype(f32)
        in_maps.append(m)

    res = bass_utils.run_bass_kernel_spmd(nc, in_maps, core_ids=list(range(NCORES)))
    _LAST_RES = res
    out = np.empty((B, S, D), np.float32)
    for b in range(B):
        o0 = res.results[2 * b]["out"]      # rank-0 shards: rows 0:128 / 256:384
        o1 = res.results[2 * b + 1]["out"]  # rank-1 shards: rows 128:256 / 384:512
        out[b, 0:128] = o0[0:128]
        out[b, 128:256] = o1[0:128]
        out[b, 256:384] = o0[128:256]
        out[b, 384:512] = o1[128:256]
    return out
